# revision 60
# baseline (speedup 1.0000x reference)
"""Barycentric-coordinates KNN kernel for Trainium2 (8 NeuronCores).

Pipeline (per core = one (batch, half-of-V) pair; 8 cores cover 4 batches x 2
halves):
  Phase 1 (device): value matrix 2q.p - |p|^2 via fp32r TensorE matmuls
    (monotone in -d^2 per query row); column index bit-packed into the low 7
    mantissa bits; DVE max8 per 128-column chunk -> 256 candidate keys/row.
  Host: decode candidate indices, exact fp32 d^2 re-rank to the true top-33
    (value asc, index asc), neighbor gather, SHOT weight normalization.
  Phase 2 (device): weighted 3x3 covariance, closed-form eigensolver (Newton
    on the characteristic cubic + cross products), SHOT sign disambiguation,
    tangent-plane log map, template-cell top-3 selection via polar-factorized
    packed keys (VAL = C + dd^2 - 2 r.(px cos + py sin), low 5 bits = k) and
    3 rounds of segmented reduce-max + suppress.
  Host: decode k-slots, gather projections, barycentric weights, assemble
    (4, 4096, 5, 8, 3, 2) output.
"""
import sys

sys.path.insert(0, "/opt/trn_rl_repo")

import numpy as np
from contextlib import ExitStack

import concourse.bass as bass
import concourse.mybir as mybir
import concourse.tile as tile
from concourse.bass_utils import run_bass_kernel_spmd
from concourse.tile import ScopedClock

f32 = np.float32
AF = mybir.ActivationFunctionType
ALU = mybir.AluOpType
DT = mybir.dt
AX = mybir.AxisListType

B, V, K = 4, 4096, 32
HALF = V // 2            # queries per core
NT = HALF // 128         # 16 v-tiles per core
NCH = 32                 # phase-1 chunk count (chunk width 128)
CAND = NCH * 8           # 256 candidates per row
R, A = 5, 8
NCELL = R * A            # 40 template cells
EPS = 1e-8
CKEY = 0.015625          # key offset: VAL = CKEY + dd^2 - 2 p.t > 0
TEMPLATE_RADIUS = 0.09
# ring radii exactly as create_template computes them in fp32
_RRJ = (f32(TEMPLATE_RADIUS)
        * (np.arange(1, R + 1, dtype=f32) / f32(R))).astype(f32)
RRJ_NEG2 = [float(v) for v in (f32(-2.0) * _RRJ).astype(f32)]

# ---------------------------------------------------------------------------
# Tile-framework workaround: walrus rejects instructions carrying more than a
# couple of sync waits. Spread extras across single-wait NOPs.
# ---------------------------------------------------------------------------


def _patched_drain_and_barrier(self, tick_clock, wait_clock):
    probe = self.nc.sync.nop(nofuse=True)
    wait_clock.add_sem_waits(probe.ins, ScopedClock({None: tick_clock.global_clock}))
    sync_info = probe.ins.sync_info
    waits = list(sync_info.on_wait or []) if sync_info is not None else []
    if len(waits) > 1:
        sync_info.on_wait = waits[:1]
        for i in range(1, len(waits)):
            extra = self.nc.sync.nop(nofuse=True)
            if extra.ins.sync_info is None:
                extra.ins.sync_info = mybir.SyncInfo(on_wait=[waits[i]], on_update=[])
            else:
                extra.ins.sync_info.on_wait = [waits[i]]
    self.nc.sync.drain()
    self.nc.all_engine_barrier()
    assert self.sems is not None
    popped = self.nc._tile_sem_poison_stack.pop()
    assert popped is self._sem_poison
    self.nc.clear_and_free_semaphores(list(self.sems.allocated().values()))
    self.nc.all_engine_barrier()


tile.TileContext._drain_and_barrier = _patched_drain_and_barrier




def split_sync_waits(nc, max_waits=1):
    for f in nc.m.functions:
        for b in f.blocks:
            new_list = []
            dirty = False
            for ins in b.instructions:
                si = ins.sync_info
                waits = list(si.on_wait) if (si is not None and si.on_wait) else []
                if len(waits) > max_waits:
                    dirty = True
                    extras, keep = waits[:-max_waits], waits[-max_waits:]
                    for j in range(0, len(extras), max_waits):
                        nop = mybir.InstNoOp(
                            name=f"I-wsplit-{nc.next_id()}", engine=ins.engine
                        )
                        nop.sync_info = mybir.SyncInfo(
                            on_wait=extras[j : j + max_waits], on_update=[]
                        )
                        new_list.append(nop)
                    si.on_wait = keep
                new_list.append(ins)
            if dirty:
                b.instructions = new_list


# ---------------------------------------------------------------------------
# Phase 1 program
# ---------------------------------------------------------------------------


def build_phase1():
    nc = bass.Bass()
    pt4 = nc.declare_dram_parameter("pt4", [4, V], DT.float32r, isOutput=False)
    qt4 = nc.declare_dram_parameter("qt4", [4, HALF], DT.float32r, isOutput=False)
    cand_o = nc.declare_dram_parameter("cand", [HALF, CAND], DT.float32, isOutput=True)

    with tile.TileContext(nc) as tc, ExitStack() as ctx:
        cpool = ctx.enter_context(tc.tile_pool(name="const", bufs=1))
        kpool = ctx.enter_context(tc.tile_pool(name="keys", bufs=3))
        opool = ctx.enter_context(tc.tile_pool(name="cand", bufs=4))
        ppool = ctx.enter_context(tc.tile_pool(name="psum", bufs=2, space="PSUM"))

        pt = cpool.tile([4, V], DT.float32r)
        qt = cpool.tile([4, HALF], DT.float32r)
        nc.sync.dma_start(pt[:], pt4[:])
        nc.sync.dma_start(qt[:], qt4[:])
        # column-in-chunk index, repeated per chunk: 0..127, 0..127, ...
        kiota = cpool.tile([128, 2048], DT.int32)
        nc.gpsimd.iota(kiota[:], pattern=[[0, 16], [1, 128]], base=0,
                       channel_multiplier=0)
        m7 = cpool.tile([128, 1], DT.int32)
        nc.vector.memset(m7[:], -128)  # 0xFFFFFF80

        for t in range(NT):
            cv = opool.tile([128, CAND], DT.float32, tag="cv")
            for jh in range(2):
                ps = ppool.tile([128, 2048], DT.float32, space="PSUM")
                for k4 in range(4):
                    nc.tensor.matmul(
                        ps[:, k4 * 512:(k4 + 1) * 512],
                        qt[:, t * 128:(t + 1) * 128],
                        pt[:, jh * 2048 + k4 * 512: jh * 2048 + (k4 + 1) * 512],
                        start=True, stop=True,
                    )
                key = kpool.tile([128, 2048], DT.int32, tag="key")
                nc.vector.scalar_tensor_tensor(
                    out=key[:], in0=ps[:].bitcast(DT.int32), scalar=m7[:],
                    in1=kiota[:], op0=ALU.bitwise_and, op1=ALU.bitwise_or)
                for c in range(16):
                    g = jh * 16 + c
                    nc.vector.max(out=cv[:, g * 8:(g + 1) * 8],
                                  in_=key[:, c * 128:(c + 1) * 128]
                                  .bitcast(DT.float32))
            nc.sync.dma_start(cand_o[t * 128:(t + 1) * 128, :], cv[:])

    split_sync_waits(nc)
    return nc


# ---------------------------------------------------------------------------
# Phase 2 program
# ---------------------------------------------------------------------------


def _register_consts(nc, values):
    for value in values:
        t = nc.alloc_sbuf_tensor(f"const-float32-{value}", [128, 1], DT.float32)
        nc.gpsimd.memset(t.ap(), value)
        nc.const_aps.aps[(DT.float32, value)] = t.ap()
    nc.all_engine_barrier()


def build_phase2():
    nc = bass.Bass()
    _register_consts(nc, [0.5, CKEY, -3.0, 64.0])
    ngh_i = nc.declare_dram_parameter("ngh", [HALF, 96], DT.float32, isOutput=False)
    cov_i = nc.declare_dram_parameter("cov6", [HALF, 8], DT.float32, isOutput=False)
    dd_i = nc.declare_dram_parameter("dd", [HALF, K], DT.float32, isOutput=False)
    tcs_i = nc.declare_dram_parameter("tcs", [128, 16], DT.float32, isOutput=False)
    rrn_i = nc.declare_dram_parameter("rrn", [128, 8], DT.float32, isOutput=False)
    m3_o = nc.declare_dram_parameter("m3o", [HALF, NCELL, 3], DT.float32,
                                     isOutput=True)
    pxy_o = nc.declare_dram_parameter("pxy", [HALF, 2, K], DT.float32,
                                      isOutput=True)

    with tile.TileContext(nc) as tc, ExitStack() as ctx:
        cp = ctx.enter_context(tc.tile_pool(name="const", bufs=1))
        sp = ctx.enter_context(tc.tile_pool(name="scratch", bufs=2))
        bp = ctx.enter_context(tc.tile_pool(name="bc", bufs=2))

        NGH = cp.tile([128, NT, 96], DT.float32)
        COV6 = cp.tile([128, NT, 8], DT.float32)
        DD = cp.tile([128, NT, K], DT.float32)
        TCS = cp.tile([128, 16], DT.float32)
        RRN = cp.tile([128, 8], DT.float32)
        nc.sync.dma_start(NGH[:], ngh_i[:].rearrange("(t p) c -> p t c", p=128))
        nc.sync.dma_start(COV6[:], cov_i[:].rearrange("(t p) c -> p t c", p=128))
        nc.sync.dma_start(DD[:], dd_i[:].rearrange("(t p) c -> p t c", p=128))
        nc.sync.dma_start(TCS[:], tcs_i[:])
        nc.sync.dma_start(RRN[:], rrn_i[:])

        # low-5-bit slot id plus the sign bit: packed keys become negative
        # floats, so max8 ranks by ascending VAL with ties to the smaller k,
        # matching the reference tie-break
        KIOTA = cp.tile([128, 2, NCELL, K], DT.int32)
        nc.gpsimd.iota(KIOTA[:], pattern=[[0, 2], [0, NCELL], [1, K]],
                       base=-2147483648, channel_multiplier=0)
        M32 = cp.tile([128, 1], DT.int32)
        nc.vector.memset(M32[:], -32)

        _tagn = [0]

        def nt_tile(pool=cp):
            _tagn[0] += 1
            return pool.tile([128, NT], DT.float32, tag=f"nt{_tagn[0]}",
                             name=f"nt{_tagn[0]}")

        def n2_tile(pool=cp):
            _tagn[0] += 1
            return pool.tile([128, 2 * NT], DT.float32, tag=f"n2{_tagn[0]}",
                             name=f"n2{_tagn[0]}")

        CXX = COV6[:, :, 0]
        CXY = COV6[:, :, 1]
        CXZ = COV6[:, :, 2]
        CYY = COV6[:, :, 3]
        CYZ = COV6[:, :, 4]
        CZZ = COV6[:, :, 5]

        # ---- eigensolver; scalar chain on (128, NT), then the two Newton
        # runs and the two eigenvector extractions merged into (128, 2*NT) ----
        def _ap(x):
            return x if isinstance(x, bass.AP) else x[:]

        def tt(dst, a, bb, op):
            nc.vector.tensor_tensor(out=_ap(dst), in0=_ap(a), in1=_ap(bb), op=op)

        def sq_act(dst, a):
            nc.scalar.activation(_ap(dst), _ap(a), AF.Square)

        Q = nt_tile()
        tt(Q, CXX, CYY, ALU.add)
        tt(Q, Q, CZZ, ALU.add)
        nc.vector.tensor_scalar_mul(Q[:], Q[:], 1.0 / 3.0)
        BXX, BYY, BZZ = nt_tile(), nt_tile(), nt_tile()
        tt(BXX, CXX, Q, ALU.subtract)
        tt(BYY, CYY, Q, ALU.subtract)
        tt(BZZ, CZZ, Q, ALU.subtract)
        P2 = nt_tile()
        T1 = nt_tile(sp)
        sq_act(P2, BXX)
        sq_act(T1, BYY)
        tt(P2, P2, T1, ALU.add)
        sq_act(T1, BZZ)
        tt(P2, P2, T1, ALU.add)
        T2 = nt_tile(sp)
        sq_act(T1, CXY)
        sq_act(T2, CXZ)
        tt(T1, T1, T2, ALU.add)
        sq_act(T2, CYZ)
        tt(T1, T1, T2, ALU.add)
        nc.vector.tensor_scalar_mul(T1[:], T1[:], 2.0)
        tt(P2, P2, T1, ALU.add)
        PP = nt_tile()
        PPX = nt_tile()
        nc.vector.tensor_scalar_mul(PPX[:], P2[:], 1.0 / 6.0)

        def polished_sqrt(dst, x, tmp):
            nc.scalar.activation(_ap(dst), _ap(x), AF.Sqrt)

        polished_sqrt(PP, PPX, T2)
        PINV = nt_tile()
        nc.vector.tensor_scalar_max(PINV[:], PP[:], 1e-20)
        nc.vector.reciprocal(PINV[:], PINV[:])
        NBXX, NBYY, NBZZ, NBXY, NBXZ, NBYZ = [nt_tile() for _ in range(6)]
        tt(NBXX, BXX, PINV, ALU.mult)
        tt(NBYY, BYY, PINV, ALU.mult)
        tt(NBZZ, BZZ, PINV, ALU.mult)
        tt(NBXY, CXY, PINV, ALU.mult)
        tt(NBXZ, CXZ, PINV, ALU.mult)
        tt(NBYZ, CYZ, PINV, ALU.mult)
        # det(B̂)
        DET = nt_tile()
        sq_act(T1, NBYZ)                     # byz^2
        tt(T2, NBYY, NBZZ, ALU.mult)
        tt(T2, T2, T1, ALU.subtract)
        tt(DET, NBXX, T2, ALU.mult)          # + bxx (byy bzz - byz^2)
        tt(T1, NBXY, NBZZ, ALU.mult)
        tt(T2, NBYZ, NBXZ, ALU.mult)
        tt(T1, T1, T2, ALU.subtract)
        tt(T1, NBXY, T1, ALU.mult)
        tt(DET, DET, T1, ALU.subtract)       # - bxy (bxy bzz - byz bxz)
        tt(T1, NBXY, NBYZ, ALU.mult)
        tt(T2, NBYY, NBXZ, ALU.mult)
        tt(T1, T1, T2, ALU.subtract)
        tt(T1, NBXZ, T1, ALU.mult)
        tt(DET, DET, T1, ALU.add)            # + bxz (bxy byz - byy bxz)
        R2 = nt_tile()                       # 2r = det  clamped to [-2, 2]
        nc.vector.tensor_scalar_min(R2[:], DET[:], 2.0)
        nc.vector.tensor_scalar_max(R2[:], R2[:], -2.0)

        # merged Newton: halves [beta(+2.2) | beta(-2.2)] over (128, 2*NT)
        def dup(src):
            d = n2_tile()
            nc.scalar.copy(d[:, 0:NT], _ap(src))
            nc.scalar.copy(d[:, NT:2 * NT], _ap(src))
            return d

        R2D = dup(R2)
        # [u | -u] for the odd-symmetry polynomial init: βmin(u) = -βmax(-u)
        R2S = n2_tile()
        nc.scalar.copy(R2S[:, 0:NT], R2[:])
        nc.scalar.activation(R2S[:, NT:2 * NT], R2[:], AF.Identity, scale=-1.0)
        T1D = n2_tile(sp)
        # cubic LS fit of the largest root of β³-3β-u on u ∈ [-2,2]
        # (max err 0.15), then 4 Newton steps -> <1e-6 away from double roots
        BETA = n2_tile()
        nc.vector.tensor_scalar(out=BETA[:], in0=R2S[:], scalar1=0.01574144,
                                scalar2=-0.03955863, op0=ALU.mult, op1=ALU.add)
        tt(BETA, BETA, R2S, ALU.mult)
        nc.vector.tensor_scalar_add(BETA[:], BETA[:], 0.15508261)
        tt(BETA, BETA, R2S, ALU.mult)
        nc.vector.tensor_scalar_add(BETA[:], BETA[:], 1.74024065)
        nc.scalar.activation(BETA[:, NT:2 * NT], BETA[:, NT:2 * NT],
                             AF.Identity, scale=-1.0)
        FV = n2_tile(sp)
        B2 = n2_tile(sp)
        for _ in range(3):
            tt(B2, BETA, BETA, ALU.mult)                  # β²
            tt(FV, B2, BETA, ALU.mult)                    # β³
            nc.vector.scalar_tensor_tensor(
                out=T1D[:], in0=BETA[:], scalar=3.0, in1=FV[:],
                op0=ALU.mult, op1=ALU.subtract)           # 3β - β³
            tt(T1D, T1D, R2D, ALU.add)                    # -f = 3β - β³ + 2r
            nc.vector.tensor_scalar(out=B2[:], in0=B2[:], scalar1=3.0,
                                    scalar2=-3.0, op0=ALU.mult, op1=ALU.add)
            nc.vector.tensor_scalar_max(B2[:], B2[:], 1e-8)
            nc.vector.reciprocal(B2[:], B2[:])
            tt(T1D, T1D, B2, ALU.mult)                    # -f/f'
            tt(BETA, BETA, T1D, ALU.add)                  # β - f/f'
        # LL = [λmax | λmin]
        PPD = dup(PP)
        QD = dup(Q)
        LL = n2_tile()
        tt(LL, PPD, BETA, ALU.mult)
        tt(LL, LL, QD, ALU.add)

        # merged eigenvector extraction: halves [x-axis(λmax) | z-axis(λmin)]
        CXXD, CXYD, CXZD = dup(CXX), dup(CXY), dup(CXZ)
        CYYD, CYZD, CZZD = dup(CYY), dup(CYZ), dup(CZZ)

        def gtt(dst, a, bb, op):
            nc.gpsimd.tensor_tensor(out=_ap(dst), in0=_ap(a), in1=_ap(bb), op=op)

        def evec2(lam):
            # columns of A - lam I; arithmetic on gpsimd (idle during the
            # eigen prologue), comparisons/reciprocals on vector, squares on
            # scalar
            D0, D1, D2 = n2_tile(sp), n2_tile(sp), n2_tile(sp)
            gtt(D0, CXXD, lam, ALU.subtract)
            gtt(D1, CYYD, lam, ALU.subtract)
            gtt(D2, CZZD, lam, ALU.subtract)
            m0 = (D0, CXYD, CXZD)
            m1 = (CXYD, D1, CYZD)
            m2 = (CXZD, CYZD, D2)

            def cross(u, v):
                rx, ry, rz = n2_tile(sp), n2_tile(sp), n2_tile(sp)
                tmp = n2_tile(sp)
                gtt(rx, u[1], v[2], ALU.mult)
                gtt(tmp, u[2], v[1], ALU.mult)
                gtt(rx, rx, tmp, ALU.subtract)
                gtt(ry, u[2], v[0], ALU.mult)
                gtt(tmp, u[0], v[2], ALU.mult)
                gtt(ry, ry, tmp, ALU.subtract)
                gtt(rz, u[0], v[1], ALU.mult)
                gtt(tmp, u[1], v[0], ALU.mult)
                gtt(rz, rz, tmp, ALU.subtract)
                return rx, ry, rz

            def norm2(c):
                n = n2_tile(sp)
                tmp = n2_tile(sp)
                sq_act(n, c[0])
                sq_act(tmp, c[1])
                gtt(n, n, tmp, ALU.add)
                sq_act(tmp, c[2])
                gtt(n, n, tmp, ALU.add)
                return n

            c01 = cross(m0, m1)
            c02 = cross(m0, m2)
            c12 = cross(m1, m2)
            n01, n02, n12 = norm2(c01), norm2(c02), norm2(c12)
            G1, G2, G3 = n2_tile(sp), n2_tile(sp), n2_tile(sp)
            tt(G1, n01, n02, ALU.is_ge)
            tt(G2, n01, n12, ALU.is_ge)
            tt(G1, G1, G2, ALU.mult)                    # pick01
            tt(G3, n02, n12, ALU.is_ge)
            U = n2_tile(sp)
            nc.vector.tensor_scalar(out=U[:], in0=G1[:], scalar1=-1.0, scalar2=1.0,
                                    op0=ALU.mult, op1=ALU.add)   # 1 - pick01
            tt(G2, U, G3, ALU.mult)                     # pick02
            nc.vector.tensor_scalar(out=G3[:], in0=G3[:], scalar1=-1.0, scalar2=1.0,
                                    op0=ALU.mult, op1=ALU.add)   # 1 - g3
            tt(G3, U, G3, ALU.mult)                     # pick12
            out = []
            for ci in range(3):
                VC = n2_tile()
                tmp = n2_tile(sp)
                gtt(VC, c01[ci], G1, ALU.mult)
                gtt(tmp, c02[ci], G2, ALU.mult)
                gtt(VC, VC, tmp, ALU.add)
                gtt(tmp, c12[ci], G3, ALU.mult)
                gtt(VC, VC, tmp, ALU.add)
                out.append(VC)
            n2v = norm2(out)
            n = n2_tile(sp)
            polished_sqrt(n, n2v, T1D)
            nc.vector.tensor_scalar_max(n[:], n[:], 1e-30)
            nc.vector.reciprocal(n[:], n[:])
            for VC in out:
                gtt(VC, VC, n, ALU.mult)
            return out

        AXD = evec2(LL)
        XAX = [v[:, 0:NT] for v in AXD]
        ZAX = [v[:, NT:2 * NT] for v in AXD]

        NGHX = NGH[:, :, 0:K]
        NGHY = NGH[:, :, K:2 * K]
        NGHZ = NGH[:, :, 2 * K:3 * K]

        def axb(t_):
            return _ap(t_).rearrange("p t -> p t ()").to_broadcast([128, NT, K])

        def dot_axis(axes, g_engine=True):
            # batched NGH . axis over all tiles; products on gpsimd, adds on vec
            DST = cp.tile([128, NT, K], DT.float32, tag=f"dot{_tagn[0]}",
                          name=f"dot{_tagn[0]}")
            _tagn[0] += 1
            TA = sp.tile([128, NT, K], DT.float32, tag="dta")
            TB = sp.tile([128, NT, K], DT.float32, tag="dtb")
            eng = nc.gpsimd if g_engine else nc.vector
            eng.tensor_tensor(out=DST[:], in0=NGHX, in1=axb(axes[0]), op=ALU.mult)
            eng.tensor_tensor(out=TA[:], in0=NGHY, in1=axb(axes[1]), op=ALU.mult)
            eng.tensor_tensor(out=TB[:], in0=NGHZ, in1=axb(axes[2]), op=ALU.mult)
            nc.vector.tensor_tensor(out=DST[:], in0=DST[:], in1=TA[:], op=ALU.add)
            nc.vector.tensor_tensor(out=DST[:], in0=DST[:], in1=TB[:], op=ALU.add)
            return DST

        # ---- disambiguation dots + signs ----
        DOTX = dot_axis(XAX, g_engine=True)
        DOTZ = dot_axis(ZAX, g_engine=False)
        SG = cp.tile([128, NT, K], DT.float32)
        FX = nt_tile()
        FZ = nt_tile()
        for DOT, F in ((DOTX, FX), (DOTZ, FZ)):
            nc.scalar.activation(SG[:], DOT[:], AF.Sign)
            nc.vector.tensor_reduce(out=F[:], in_=SG[:], axis=AX.X, op=ALU.add)
            nc.scalar.activation(F[:], F[:], AF.Sign, bias=0.5, scale=1.0)
        for c in range(3):
            tt(XAX[c], XAX[c], FX, ALU.mult)
            tt(ZAX[c], ZAX[c], FZ, ALU.mult)
        nc.vector.tensor_tensor(out=DOTX[:], in0=DOTX[:], in1=axb(FX), op=ALU.mult)
        # y = cross(z, x)
        YAX = []
        for (i1, i2) in ((1, 2), (2, 0), (0, 1)):
            YC = nt_tile()
            YT = nt_tile(sp)
            gtt(YC, ZAX[i1], XAX[i2], ALU.mult)
            gtt(YT, ZAX[i2], XAX[i1], ALU.mult)
            gtt(YC, YC, YT, ALU.subtract)
            YAX.append(YC)
        DOTY = dot_axis(YAX, g_engine=True)

        # ---- projections (batched over all tiles) into PXY ----
        PXY = cp.tile([128, NT, 2, K], DT.float32)
        PXs = PXY[:, :, 0, :]
        PYs = PXY[:, :, 1, :]
        SC = cp.tile([128, NT, K], DT.float32)
        U2 = cp.tile([128, NT, K], DT.float32)
        RCN = cp.tile([128, NT, K], DT.float32)
        nc.scalar.activation(U2[:], DOTX[:], AF.Square)
        nc.scalar.activation(RCN[:], DOTY[:], AF.Square)
        nc.vector.tensor_tensor(out=U2[:], in0=U2[:], in1=RCN[:], op=ALU.add)
        nc.scalar.activation(SC[:], U2[:], AF.Sqrt)
        nc.vector.tensor_scalar_add(SC[:], SC[:], EPS)
        nc.vector.reciprocal(SC[:], SC[:])
        nc.vector.tensor_tensor(out=SC[:], in0=SC[:], in1=DD[:], op=ALU.mult)
        nc.vector.tensor_tensor(out=PXs, in0=DOTX[:], in1=SC[:], op=ALU.mult)
        nc.vector.tensor_tensor(out=PYs, in0=DOTY[:], in1=SC[:], op=ALU.mult)
        nc.sync.dma_start(pxy_o[:].rearrange("(t p) x k -> p t x k", p=128), PXY[:])

        # PPC = dd^2 + CKEY; with VM = -2r.U the key VAL = CKEY + d2d^2 - |t|^2
        # is strictly positive and fine-grained near the top (small) end
        PPC = cp.tile([128, NT, K], DT.float32)
        nc.scalar.activation(PPC[:], DD[:], AF.Square)
        nc.scalar.activation(PPC[:], PPC[:], AF.Identity, bias=CKEY, scale=1.0)

        # ---- BC selection, 2 v-tiles per step ----
        # Keys are positive (VAL = CKEY + d2d^2 - |t|^2 packed with k in the
        # low 5 bits); the 3 nearest are 3 rounds of segmented reduce-MIN.
        # The keep-mask is_le(KEY, min) doubles as the suppress addend
        # (+1.0 at the min pushes it above every real key < 0.25).
        # Emission is software-pipelined: gpsimd value-chains for a group of
        # steps first, then rounds pair-interleaved to hide cross-engine
        # latency (engine queues are in-order).
        TP = 2
        NS = NT // TP
        COSB = TCS[:, 0:8].rearrange("p a -> p () a ()") \
            .to_broadcast([128, TP, A, K])
        SINB = TCS[:, 8:16].rearrange("p a -> p () a ()") \
            .to_broadcast([128, TP, A, K])

        def emit_chain(st):
            t = st * TP
            pxb = PXY[:, t:t + TP, 0, :].rearrange("p t k -> p t () k") \
                .to_broadcast([128, TP, A, K])
            pyb = PXY[:, t:t + TP, 1, :].rearrange("p t k -> p t () k") \
                .to_broadcast([128, TP, A, K])
            T1B = bp.tile([128, TP, A, K], DT.float32, tag="t1b")
            T2B = bp.tile([128, TP, A, K], DT.float32, tag="t2b")
            nc.gpsimd.tensor_tensor(out=T1B[:], in0=pxb, in1=COSB, op=ALU.mult)
            nc.gpsimd.tensor_tensor(out=T2B[:], in0=pyb, in1=SINB, op=ALU.mult)
            UT = bp.tile([128, TP, A, K], DT.float32, tag="ut")
            nc.gpsimd.tensor_tensor(out=UT[:], in0=T1B[:], in1=T2B[:],
                                    op=ALU.add)
            # radius expansion on the (otherwise idle) scalar engine:
            # VALT[.., r-ring, ..] = -2 r_j . U
            VALT = bp.tile([128, TP, NCELL, K], DT.float32, tag="val", bufs=4)
            for r in range(R):
                nc.scalar.activation(
                    VALT[:, :, r * A:(r + 1) * A, :], UT[:], AF.Identity,
                    scale=RRJ_NEG2[r])
            ppcb = PPC[:, t:t + TP, :].rearrange("p t k -> p t () k") \
                .to_broadcast([128, TP, NCELL, K])
            nc.gpsimd.tensor_tensor(out=VALT[:], in0=VALT[:], in1=ppcb,
                                    op=ALU.add)
            return VALT

        def emit_select(st, VALT):
            # bit-pack in place (low 5 mantissa bits -> slot id, sign set),
            # then top-3 per cell directly via max8 — one op per cell, no
            # cross-op dependencies
            t = st * TP
            nc.vector.scalar_tensor_tensor(
                out=VALT[:].bitcast(DT.int32), in0=VALT[:].bitcast(DT.int32),
                scalar=M32[:], in1=KIOTA[:], op0=ALU.bitwise_and,
                op1=ALU.bitwise_or)
            M8 = bp.tile([128, TP, NCELL, 8], DT.float32, tag="m8", bufs=4)
            keyv = VALT[:].bitcast(DT.float32)
            for tt_ in range(TP):
                for c in range(NCELL):
                    nc.vector.max(out=M8[:, tt_, c, :], in_=keyv[:, tt_, c, :])
            M3C = bp.tile([128, TP, NCELL, 3], DT.float32, tag="m3c", bufs=4)
            nc.scalar.copy(M3C[:], M8[:, :, :, 0:3])
            nc.sync.dma_start(
                m3_o[t * 128:(t + TP) * 128, :, :]
                .rearrange("(t p) c s -> p t c s", p=128), M3C[:])

        # stream gp chains ahead; vector packs+max8s naturally pipeline
        states = {st: emit_chain(st) for st in range(3)}
        for st in range(NS):
            if st + 3 < NS:
                states[st + 3] = emit_chain(st + 3)
            emit_select(st, states[st])

    split_sync_waits(nc)
    return nc


# ---------------------------------------------------------------------------
# Host glue
# ---------------------------------------------------------------------------


def host_prep_phase1(vertices):
    """vertices (4, 4096, 3) -> list of 8 input maps."""
    maps = []
    for core in range(8):
        b, h = core // 2, core % 2
        verts = np.ascontiguousarray(vertices[b], dtype=f32)
        sq = (verts * verts).sum(-1, dtype=f32).astype(f32)
        pt4 = np.concatenate([verts.T, sq[None, :]], axis=0).astype(f32)
        Q = verts[h * HALF:(h + 1) * HALF]
        qt4 = np.concatenate([2.0 * Q.T, -np.ones((1, HALF), f32)],
                             axis=0).astype(f32)
        maps.append({"pt4": np.ascontiguousarray(pt4),
                     "qt4": np.ascontiguousarray(qt4)})
    return maps


_CHUNK_BASE = (np.arange(CAND, dtype=np.uint32) // 8 * 128)


def host_merge(cand_bits, verts, h):
    """Decode chunk candidates, exact fp32 top-33 re-rank.

    -> nbr (HALF,32) int64, d (HALF,32), radius (HALF,)."""
    bits = cand_bits.view(np.uint32)
    cand = ((bits & 127) + _CHUNK_BASE[None, :]).astype(np.int64)  # (HALF, 256)
    sq = (verts * verts).sum(-1, dtype=f32).astype(f32)
    Q = verts[h * HALF:(h + 1) * HALF]
    qsq = sq[h * HALF:(h + 1) * HALF]
    pc = verts[cand]                                               # (HALF,256,3)
    dots = np.einsum("qck,qk->qc", pc, Q, dtype=f32).astype(f32)
    d2 = (sq[cand] + qsq[:, None] - 2.0 * dots).astype(f32)
    order = np.lexsort((cand, d2), axis=1)[:, :33]
    top = np.take_along_axis(cand, order, axis=1)
    d2t = np.take_along_axis(d2, order, axis=1)
    d33 = np.sqrt(np.maximum(d2t, 0.0)).astype(f32)
    return top[:, :32], d33[:, :32], d33[:, 32]


def host_prep_phase2(vertices, template, p1_results):
    """Build phase-2 input maps + per-core nbr tables from phase-1 outputs."""
    template = np.asarray(template, f32)
    tx = template[..., 0]                    # (R, A)
    ty = template[..., 1]
    rr = tx[:, 0].astype(f32)                # angle 0: sin=0, cos=1 -> r_j
    assert np.array_equal(rr, _RRJ), "template radii differ from compiled-in"
    cosv = (tx[0] / rr[0]).astype(f32)
    sinv = (ty[0] / rr[0]).astype(f32)
    tcs = np.ascontiguousarray(np.broadcast_to(
        np.concatenate([cosv, sinv])[None, :], (128, 16))).astype(f32)
    rrn = np.zeros(8, f32)
    rrn[:R] = -2.0 * rr
    rrn = np.ascontiguousarray(np.broadcast_to(rrn[None, :], (128, 8))).astype(f32)
    maps, nbrs = [], []
    for core in range(8):
        b, h = core // 2, core % 2
        verts = np.ascontiguousarray(vertices[b], dtype=f32)
        nbr, d, radius = host_merge(p1_results[core]["cand"], verts, h)
        Q = verts[h * HALF:(h + 1) * HALF]
        neigh = (verts[nbr] - Q[:, None, :]).astype(f32)          # (HALF, 32, 3)
        ngh = np.ascontiguousarray(neigh.transpose(0, 2, 1).reshape(HALF, 96))
        w = (radius[:, None] - d).astype(f32)
        nw = (neigh * w[:, :, None]).astype(f32)
        cov = np.matmul(nw.transpose(0, 2, 1), neigh).astype(f32)  # (HALF, 3, 3)
        cov /= (w.sum(1, dtype=f32)[:, None, None] + f32(EPS))
        cov6 = np.zeros((HALF, 8), f32)
        cov6[:, 0] = cov[:, 0, 0]
        cov6[:, 1] = cov[:, 0, 1]
        cov6[:, 2] = cov[:, 0, 2]
        cov6[:, 3] = cov[:, 1, 1]
        cov6[:, 4] = cov[:, 1, 2]
        cov6[:, 5] = cov[:, 2, 2]
        maps.append({"ngh": ngh, "cov6": cov6, "dd": np.ascontiguousarray(d),
                     "tcs": tcs, "rrn": rrn})
        nbrs.append(nbr)
    return maps, nbrs


def host_assemble(p2_results, nbrs, template):
    """Decode k-slots, gather projections, barycentric weights, assemble."""
    template = np.asarray(template, f32)
    tmx = template[..., 0].reshape(NCELL).astype(f32)
    tmy = template[..., 1].reshape(NCELL).astype(f32)
    out = np.zeros((B, V, R, A, 3, 2), f32)
    rows = np.arange(HALF)[:, None, None]
    for core in range(8):
        b, h = core // 2, core % 2
        m3 = np.ascontiguousarray(p2_results[core]["m3o"])        # (HALF, 40, 3)
        k3 = (m3.view(np.int32) & 31).astype(np.int64)            # (HALF, 40, 3)
        pxy = p2_results[core]["pxy"]                             # (HALF, 2, 32)
        px = np.ascontiguousarray(pxy[:, 0, :])
        py = np.ascontiguousarray(pxy[:, 1, :])
        pxs = px[rows, k3]                                        # (HALF, 40, 3)
        pys = py[rows, k3]
        p0x, p1x, p2x = pxs[..., 0], pxs[..., 1], pxs[..., 2]
        p0y, p1y, p2y = pys[..., 0], pys[..., 1], pys[..., 2]
        v0x, v0y = p2x - p0x, p2y - p0y
        v1x, v1y = p1x - p0x, p1y - p0y
        v2x, v2y = tmx[None, :] - p0x, tmy[None, :] - p0y
        d00 = v0x * v0x + v0y * v0y
        d01 = v0x * v1x + v0y * v1y
        d02 = v0x * v2x + v0y * v2y
        d11 = v1x * v1x + v1y * v1y
        d12 = v1x * v2x + v1y * v2y
        den = d00 * d11 - d01 * d01 + f32(1e-6)
        w2 = (d11 * d02 - d01 * d12) / den
        w1 = (d00 * d12 - d01 * d02) / den
        w0 = f32(1.0) - w2 - w1
        weights = np.stack([w2, w1, w0], axis=-1)                 # (HALF, 40, 3)
        pidx = nbrs[core][rows[..., 0], k3.reshape(HALF, -1)].reshape(HALF, NCELL, 3)
        sl = slice(h * HALF, (h + 1) * HALF)
        out[b, sl, ..., 0] = pidx.reshape(HALF, R, A, 3).astype(f32)
        out[b, sl, ..., 1] = weights.reshape(HALF, R, A, 3).astype(f32)
    return out


_PROGS = {}


def _prog(name):
    if name not in _PROGS:
        _PROGS[name] = build_phase1() if name == "p1" else build_phase2()
    return _PROGS[name]


def run_phase1(vertices, trace=False):
    maps = host_prep_phase1(vertices)
    return run_bass_kernel_spmd(_prog("p1"), maps, list(range(8)), trace=trace)


def kernel(vertices, template, trace=False, _timing=None):
    vertices = np.asarray(vertices, f32)
    template = np.asarray(template, f32)
    r1 = run_bass_kernel_spmd(_prog("p1"), host_prep_phase1(vertices),
                              list(range(8)), trace=trace)
    maps2, nbrs = host_prep_phase2(vertices, template, r1.results)
    r2 = run_bass_kernel_spmd(_prog("p2"), maps2, list(range(8)), trace=trace)
    if _timing is not None:
        _timing["phase1"] = r1
        _timing["phase2"] = r2
        _timing["maps2"] = maps2
        _timing["nbrs"] = nbrs
    return host_assemble(r2.results, nbrs, template)


if __name__ == "__main__":
    # Phase-1 standalone check: exact top-33 coverage vs numpy brute force.
    cache = np.load("/root/problem/dev_cache/ref.npz")
    vertices = cache["vertices"]
    res = run_phase1(vertices)
    nbad = 0
    for core in range(8):
        b, h = core // 2, core % 2
        verts = np.ascontiguousarray(vertices[b], dtype=f32)
        nbr, d, rad = host_merge(res.results[core]["cand"], verts, h)
        # numpy exact reference
        sq = (verts * verts).sum(-1, dtype=f32).astype(f32)
        Q = verts[h * HALF:(h + 1) * HALF]
        d2full = (sq[None, :] + sq[h * HALF:(h + 1) * HALF, None]
                  - 2.0 * (Q @ verts.T)).astype(f32)
        order = np.lexsort((np.broadcast_to(np.arange(V), d2full.shape), d2full),
                           axis=1)[:, :33]
        miss = (np.sort(nbr, 1) != np.sort(order[:, :32], 1)).sum()
        print(f"core {core}: top32 mismatches={miss}")
        nbad += miss
    print("total nbr mismatches vs numpy exact:", nbad)


# revision 61
# speedup vs baseline: 1.1582x; 1.1582x over previous
"""Barycentric-coordinates KNN kernel for Trainium2 (8 NeuronCores).

Pipeline (per core = one (batch, half-of-V) pair; 8 cores cover 4 batches x 2
halves):
  Phase 1 (device): value matrix 2q.p - |p|^2 via fp32r TensorE matmuls
    (monotone in -d^2 per query row); column index bit-packed into the low 7
    mantissa bits; DVE max8 per 128-column chunk -> 256 candidate keys/row.
  Host: decode candidate indices, exact fp32 d^2 re-rank to the true top-33
    (value asc, index asc), neighbor gather, SHOT weight normalization.
  Phase 2 (device): weighted 3x3 covariance, closed-form eigensolver (Newton
    on the characteristic cubic + cross products), SHOT sign disambiguation,
    tangent-plane log map, template-cell top-3 selection via polar-factorized
    packed keys (VAL = C + dd^2 - 2 r.(px cos + py sin), low 5 bits = k) and
    3 rounds of segmented reduce-max + suppress.
  Host: decode k-slots, gather projections, barycentric weights, assemble
    (4, 4096, 5, 8, 3, 2) output.
"""
import sys

sys.path.insert(0, "/opt/trn_rl_repo")

import numpy as np
from contextlib import ExitStack

import concourse.bass as bass
import concourse.mybir as mybir
import concourse.tile as tile
from concourse.bass_utils import run_bass_kernel_spmd
from concourse.tile import ScopedClock

f32 = np.float32
AF = mybir.ActivationFunctionType
ALU = mybir.AluOpType
DT = mybir.dt
AX = mybir.AxisListType

B, V, K = 4, 4096, 32
HALF = V // 2            # queries per core
NT = HALF // 128         # 16 v-tiles per core
NCH = 32                 # phase-1 chunk count (chunk width 128)
CAND = NCH * 8           # 256 candidates per row
R, A = 5, 8
NCELL = R * A            # 40 template cells
EPS = 1e-8
CKEY = 0.015625          # key offset: VAL = CKEY + dd^2 - 2 p.t > 0
TEMPLATE_RADIUS = 0.09
# ring radii exactly as create_template computes them in fp32
_RRJ = (f32(TEMPLATE_RADIUS)
        * (np.arange(1, R + 1, dtype=f32) / f32(R))).astype(f32)
RRJ_NEG2 = [float(v) for v in (f32(-2.0) * _RRJ).astype(f32)]

# ---------------------------------------------------------------------------
# Tile-framework workaround: walrus rejects instructions carrying more than a
# couple of sync waits. Spread extras across single-wait NOPs.
# ---------------------------------------------------------------------------


def _patched_drain_and_barrier(self, tick_clock, wait_clock):
    probe = self.nc.sync.nop(nofuse=True)
    wait_clock.add_sem_waits(probe.ins, ScopedClock({None: tick_clock.global_clock}))
    sync_info = probe.ins.sync_info
    waits = list(sync_info.on_wait or []) if sync_info is not None else []
    if len(waits) > 1:
        sync_info.on_wait = waits[:1]
        for i in range(1, len(waits)):
            extra = self.nc.sync.nop(nofuse=True)
            if extra.ins.sync_info is None:
                extra.ins.sync_info = mybir.SyncInfo(on_wait=[waits[i]], on_update=[])
            else:
                extra.ins.sync_info.on_wait = [waits[i]]
    self.nc.sync.drain()
    self.nc.all_engine_barrier()
    assert self.sems is not None
    popped = self.nc._tile_sem_poison_stack.pop()
    assert popped is self._sem_poison
    self.nc.clear_and_free_semaphores(list(self.sems.allocated().values()))
    self.nc.all_engine_barrier()


tile.TileContext._drain_and_barrier = _patched_drain_and_barrier




def split_sync_waits(nc, max_waits=1):
    for f in nc.m.functions:
        for b in f.blocks:
            new_list = []
            dirty = False
            for ins in b.instructions:
                si = ins.sync_info
                waits = list(si.on_wait) if (si is not None and si.on_wait) else []
                if len(waits) > max_waits:
                    dirty = True
                    extras, keep = waits[:-max_waits], waits[-max_waits:]
                    for j in range(0, len(extras), max_waits):
                        nop = mybir.InstNoOp(
                            name=f"I-wsplit-{nc.next_id()}", engine=ins.engine
                        )
                        nop.sync_info = mybir.SyncInfo(
                            on_wait=extras[j : j + max_waits], on_update=[]
                        )
                        new_list.append(nop)
                    si.on_wait = keep
                new_list.append(ins)
            if dirty:
                b.instructions = new_list


# ---------------------------------------------------------------------------
# Phase 1 program
# ---------------------------------------------------------------------------


def build_phase1():
    nc = bass.Bass()
    pt4 = nc.declare_dram_parameter("pt4", [4, V], DT.float32r, isOutput=False)
    qt4 = nc.declare_dram_parameter("qt4", [4, HALF], DT.float32r, isOutput=False)
    cand_o = nc.declare_dram_parameter("cand", [HALF, CAND], DT.float32, isOutput=True)

    with tile.TileContext(nc) as tc, ExitStack() as ctx:
        cpool = ctx.enter_context(tc.tile_pool(name="const", bufs=1))
        kpool = ctx.enter_context(tc.tile_pool(name="keys", bufs=3))
        opool = ctx.enter_context(tc.tile_pool(name="cand", bufs=4))
        ppool = ctx.enter_context(tc.tile_pool(name="psum", bufs=2, space="PSUM"))

        pt = cpool.tile([4, V], DT.float32r)
        qt = cpool.tile([4, HALF], DT.float32r)
        nc.sync.dma_start(pt[:], pt4[:])
        nc.sync.dma_start(qt[:], qt4[:])
        # column-in-chunk index, repeated per chunk: 0..127, 0..127, ...
        kiota = cpool.tile([128, 2048], DT.int32)
        nc.gpsimd.iota(kiota[:], pattern=[[0, 16], [1, 128]], base=0,
                       channel_multiplier=0)
        m7 = cpool.tile([128, 1], DT.int32)
        nc.vector.memset(m7[:], -128)  # 0xFFFFFF80

        for t in range(NT):
            cv = opool.tile([128, CAND], DT.float32, tag="cv")
            for jh in range(2):
                ps = ppool.tile([128, 2048], DT.float32, space="PSUM")
                for k4 in range(4):
                    nc.tensor.matmul(
                        ps[:, k4 * 512:(k4 + 1) * 512],
                        qt[:, t * 128:(t + 1) * 128],
                        pt[:, jh * 2048 + k4 * 512: jh * 2048 + (k4 + 1) * 512],
                        start=True, stop=True,
                    )
                key = kpool.tile([128, 2048], DT.int32, tag="key")
                nc.vector.scalar_tensor_tensor(
                    out=key[:], in0=ps[:].bitcast(DT.int32), scalar=m7[:],
                    in1=kiota[:], op0=ALU.bitwise_and, op1=ALU.bitwise_or)
                for c in range(16):
                    g = jh * 16 + c
                    nc.vector.max(out=cv[:, g * 8:(g + 1) * 8],
                                  in_=key[:, c * 128:(c + 1) * 128]
                                  .bitcast(DT.float32))
            nc.sync.dma_start(cand_o[t * 128:(t + 1) * 128, :], cv[:])

    split_sync_waits(nc)
    return nc


# ---------------------------------------------------------------------------
# Phase 2 program
# ---------------------------------------------------------------------------


def _register_consts(nc, values):
    for value in values:
        t = nc.alloc_sbuf_tensor(f"const-float32-{value}", [128, 1], DT.float32)
        nc.gpsimd.memset(t.ap(), value)
        nc.const_aps.aps[(DT.float32, value)] = t.ap()
    nc.all_engine_barrier()


def build_phase2():
    nc = bass.Bass()
    _register_consts(nc, [0.5, CKEY, -3.0, 64.0])
    ngh_i = nc.declare_dram_parameter("ngh", [HALF, 96], DT.float32, isOutput=False)
    cov_i = nc.declare_dram_parameter("cov6", [HALF, 8], DT.float32, isOutput=False)
    dd_i = nc.declare_dram_parameter("dd", [HALF, K], DT.float32, isOutput=False)
    tcs_i = nc.declare_dram_parameter("tcs", [128, 16], DT.float32, isOutput=False)
    rrn_i = nc.declare_dram_parameter("rrn", [128, 8], DT.float32, isOutput=False)
    m3_o = nc.declare_dram_parameter("m3o", [HALF, NCELL, 3], DT.float32,
                                     isOutput=True)
    pxy_o = nc.declare_dram_parameter("pxy", [HALF, 2, K], DT.float32,
                                      isOutput=True)

    with tile.TileContext(nc) as tc, ExitStack() as ctx:
        cp = ctx.enter_context(tc.tile_pool(name="const", bufs=1))
        sp = ctx.enter_context(tc.tile_pool(name="scratch", bufs=2))
        bp = ctx.enter_context(tc.tile_pool(name="bc", bufs=2))

        NGH = cp.tile([128, NT, 96], DT.float32)
        COV6 = cp.tile([128, NT, 8], DT.float32)
        DD = cp.tile([128, NT, K], DT.float32)
        TCS = cp.tile([128, 16], DT.float32)
        RRN = cp.tile([128, 8], DT.float32)
        nc.sync.dma_start(NGH[:], ngh_i[:].rearrange("(t p) c -> p t c", p=128))
        nc.sync.dma_start(COV6[:], cov_i[:].rearrange("(t p) c -> p t c", p=128))
        nc.sync.dma_start(DD[:], dd_i[:].rearrange("(t p) c -> p t c", p=128))
        nc.sync.dma_start(TCS[:], tcs_i[:])
        nc.sync.dma_start(RRN[:], rrn_i[:])

        # low-5-bit slot id plus the sign bit: packed keys become negative
        # floats, so max8 ranks by ascending VAL with ties to the smaller k,
        # matching the reference tie-break
        KIOTA = cp.tile([128, 2, NCELL, K], DT.int32)
        nc.gpsimd.iota(KIOTA[:], pattern=[[0, 2], [0, NCELL], [1, K]],
                       base=-2147483648, channel_multiplier=0)
        M32 = cp.tile([128, 1], DT.int32)
        nc.vector.memset(M32[:], -32)

        _tagn = [0]

        def nt_tile(pool=cp):
            _tagn[0] += 1
            return pool.tile([128, NT], DT.float32, tag=f"nt{_tagn[0]}",
                             name=f"nt{_tagn[0]}")

        def n2_tile(pool=cp):
            _tagn[0] += 1
            return pool.tile([128, 2 * NT], DT.float32, tag=f"n2{_tagn[0]}",
                             name=f"n2{_tagn[0]}")

        CXX = COV6[:, :, 0]
        CXY = COV6[:, :, 1]
        CXZ = COV6[:, :, 2]
        CYY = COV6[:, :, 3]
        CYZ = COV6[:, :, 4]
        CZZ = COV6[:, :, 5]

        # ---- eigensolver; scalar chain on (128, NT), then the two Newton
        # runs and the two eigenvector extractions merged into (128, 2*NT) ----
        def _ap(x):
            return x if isinstance(x, bass.AP) else x[:]

        def tt(dst, a, bb, op):
            nc.vector.tensor_tensor(out=_ap(dst), in0=_ap(a), in1=_ap(bb), op=op)

        def sq_act(dst, a):
            nc.scalar.activation(_ap(dst), _ap(a), AF.Square)

        Q = nt_tile()
        tt(Q, CXX, CYY, ALU.add)
        tt(Q, Q, CZZ, ALU.add)
        nc.vector.tensor_scalar_mul(Q[:], Q[:], 1.0 / 3.0)
        BXX, BYY, BZZ = nt_tile(), nt_tile(), nt_tile()
        tt(BXX, CXX, Q, ALU.subtract)
        tt(BYY, CYY, Q, ALU.subtract)
        tt(BZZ, CZZ, Q, ALU.subtract)
        P2 = nt_tile()
        T1 = nt_tile(sp)
        sq_act(P2, BXX)
        sq_act(T1, BYY)
        tt(P2, P2, T1, ALU.add)
        sq_act(T1, BZZ)
        tt(P2, P2, T1, ALU.add)
        T2 = nt_tile(sp)
        sq_act(T1, CXY)
        sq_act(T2, CXZ)
        tt(T1, T1, T2, ALU.add)
        sq_act(T2, CYZ)
        tt(T1, T1, T2, ALU.add)
        nc.vector.tensor_scalar_mul(T1[:], T1[:], 2.0)
        tt(P2, P2, T1, ALU.add)
        PP = nt_tile()
        PPX = nt_tile()
        nc.vector.tensor_scalar_mul(PPX[:], P2[:], 1.0 / 6.0)

        def polished_sqrt(dst, x, tmp):
            # ACT Sqrt is ~7e-6; one Newton step s' = (s + x/s)/2 fixes it
            nc.scalar.activation(_ap(dst), _ap(x), AF.Sqrt)
            nc.vector.tensor_scalar_max(_ap(tmp), _ap(dst), 1e-30)
            nc.vector.reciprocal(_ap(tmp), _ap(tmp))
            nc.vector.tensor_tensor(out=_ap(tmp), in0=_ap(x), in1=_ap(tmp),
                                    op=ALU.mult)
            nc.vector.tensor_tensor(out=_ap(dst), in0=_ap(dst), in1=_ap(tmp),
                                    op=ALU.add)
            nc.vector.tensor_scalar_mul(_ap(dst), _ap(dst), 0.5)

        polished_sqrt(PP, PPX, T2)
        PINV = nt_tile()
        nc.vector.tensor_scalar_max(PINV[:], PP[:], 1e-20)
        nc.vector.reciprocal(PINV[:], PINV[:])
        NBXX, NBYY, NBZZ, NBXY, NBXZ, NBYZ = [nt_tile() for _ in range(6)]
        tt(NBXX, BXX, PINV, ALU.mult)
        tt(NBYY, BYY, PINV, ALU.mult)
        tt(NBZZ, BZZ, PINV, ALU.mult)
        tt(NBXY, CXY, PINV, ALU.mult)
        tt(NBXZ, CXZ, PINV, ALU.mult)
        tt(NBYZ, CYZ, PINV, ALU.mult)
        # det(B̂)
        DET = nt_tile()
        sq_act(T1, NBYZ)                     # byz^2
        tt(T2, NBYY, NBZZ, ALU.mult)
        tt(T2, T2, T1, ALU.subtract)
        tt(DET, NBXX, T2, ALU.mult)          # + bxx (byy bzz - byz^2)
        tt(T1, NBXY, NBZZ, ALU.mult)
        tt(T2, NBYZ, NBXZ, ALU.mult)
        tt(T1, T1, T2, ALU.subtract)
        tt(T1, NBXY, T1, ALU.mult)
        tt(DET, DET, T1, ALU.subtract)       # - bxy (bxy bzz - byz bxz)
        tt(T1, NBXY, NBYZ, ALU.mult)
        tt(T2, NBYY, NBXZ, ALU.mult)
        tt(T1, T1, T2, ALU.subtract)
        tt(T1, NBXZ, T1, ALU.mult)
        tt(DET, DET, T1, ALU.add)            # + bxz (bxy byz - byy bxz)
        R2 = nt_tile()                       # 2r = det  clamped to [-2, 2]
        nc.vector.tensor_scalar_min(R2[:], DET[:], 2.0)
        nc.vector.tensor_scalar_max(R2[:], R2[:], -2.0)

        # merged Newton: halves [beta(+2.2) | beta(-2.2)] over (128, 2*NT)
        def dup(src):
            d = n2_tile()
            nc.scalar.copy(d[:, 0:NT], _ap(src))
            nc.scalar.copy(d[:, NT:2 * NT], _ap(src))
            return d

        R2D = dup(R2)
        # [u | -u] for the odd-symmetry polynomial init: βmin(u) = -βmax(-u)
        R2S = n2_tile()
        nc.scalar.copy(R2S[:, 0:NT], R2[:])
        nc.scalar.activation(R2S[:, NT:2 * NT], R2[:], AF.Identity, scale=-1.0)
        T1D = n2_tile(sp)
        # cubic LS fit of the largest root of β³-3β-u on u ∈ [-2,2]
        # (max err 0.15), then 4 Newton steps -> <1e-6 away from double roots
        BETA = n2_tile()
        nc.vector.tensor_scalar(out=BETA[:], in0=R2S[:], scalar1=0.01574144,
                                scalar2=-0.03955863, op0=ALU.mult, op1=ALU.add)
        tt(BETA, BETA, R2S, ALU.mult)
        nc.vector.tensor_scalar_add(BETA[:], BETA[:], 0.15508261)
        tt(BETA, BETA, R2S, ALU.mult)
        nc.vector.tensor_scalar_add(BETA[:], BETA[:], 1.74024065)
        nc.scalar.activation(BETA[:, NT:2 * NT], BETA[:, NT:2 * NT],
                             AF.Identity, scale=-1.0)
        FV = n2_tile(sp)
        B2 = n2_tile(sp)
        for _ in range(4):
            tt(B2, BETA, BETA, ALU.mult)                  # β²
            tt(FV, B2, BETA, ALU.mult)                    # β³
            nc.vector.scalar_tensor_tensor(
                out=T1D[:], in0=BETA[:], scalar=3.0, in1=FV[:],
                op0=ALU.mult, op1=ALU.subtract)           # 3β - β³
            tt(T1D, T1D, R2D, ALU.add)                    # -f = 3β - β³ + 2r
            nc.vector.tensor_scalar(out=B2[:], in0=B2[:], scalar1=3.0,
                                    scalar2=-3.0, op0=ALU.mult, op1=ALU.add)
            nc.vector.tensor_scalar_max(B2[:], B2[:], 1e-8)
            nc.vector.reciprocal(B2[:], B2[:])
            tt(T1D, T1D, B2, ALU.mult)                    # -f/f'
            tt(BETA, BETA, T1D, ALU.add)                  # β - f/f'
        # LL = [λmax | λmin]
        PPD = dup(PP)
        QD = dup(Q)
        LL = n2_tile()
        tt(LL, PPD, BETA, ALU.mult)
        tt(LL, LL, QD, ALU.add)

        # merged eigenvector extraction: halves [x-axis(λmax) | z-axis(λmin)]
        CXXD, CXYD, CXZD = dup(CXX), dup(CXY), dup(CXZ)
        CYYD, CYZD, CZZD = dup(CYY), dup(CYZ), dup(CZZ)

        def gtt(dst, a, bb, op):
            nc.gpsimd.tensor_tensor(out=_ap(dst), in0=_ap(a), in1=_ap(bb), op=op)

        def evec2(lam):
            # columns of A - lam I; arithmetic on gpsimd (idle during the
            # eigen prologue), comparisons/reciprocals on vector, squares on
            # scalar
            D0, D1, D2 = n2_tile(sp), n2_tile(sp), n2_tile(sp)
            gtt(D0, CXXD, lam, ALU.subtract)
            gtt(D1, CYYD, lam, ALU.subtract)
            gtt(D2, CZZD, lam, ALU.subtract)
            m0 = (D0, CXYD, CXZD)
            m1 = (CXYD, D1, CYZD)
            m2 = (CXZD, CYZD, D2)

            def cross(u, v):
                rx, ry, rz = n2_tile(sp), n2_tile(sp), n2_tile(sp)
                tmp = n2_tile(sp)
                gtt(rx, u[1], v[2], ALU.mult)
                gtt(tmp, u[2], v[1], ALU.mult)
                gtt(rx, rx, tmp, ALU.subtract)
                gtt(ry, u[2], v[0], ALU.mult)
                gtt(tmp, u[0], v[2], ALU.mult)
                gtt(ry, ry, tmp, ALU.subtract)
                gtt(rz, u[0], v[1], ALU.mult)
                gtt(tmp, u[1], v[0], ALU.mult)
                gtt(rz, rz, tmp, ALU.subtract)
                return rx, ry, rz

            def norm2(c):
                n = n2_tile(sp)
                tmp = n2_tile(sp)
                sq_act(n, c[0])
                sq_act(tmp, c[1])
                gtt(n, n, tmp, ALU.add)
                sq_act(tmp, c[2])
                gtt(n, n, tmp, ALU.add)
                return n

            c01 = cross(m0, m1)
            c02 = cross(m0, m2)
            c12 = cross(m1, m2)
            n01, n02, n12 = norm2(c01), norm2(c02), norm2(c12)
            G1, G2, G3 = n2_tile(sp), n2_tile(sp), n2_tile(sp)
            tt(G1, n01, n02, ALU.is_ge)
            tt(G2, n01, n12, ALU.is_ge)
            tt(G1, G1, G2, ALU.mult)                    # pick01
            tt(G3, n02, n12, ALU.is_ge)
            U = n2_tile(sp)
            nc.vector.tensor_scalar(out=U[:], in0=G1[:], scalar1=-1.0, scalar2=1.0,
                                    op0=ALU.mult, op1=ALU.add)   # 1 - pick01
            tt(G2, U, G3, ALU.mult)                     # pick02
            nc.vector.tensor_scalar(out=G3[:], in0=G3[:], scalar1=-1.0, scalar2=1.0,
                                    op0=ALU.mult, op1=ALU.add)   # 1 - g3
            tt(G3, U, G3, ALU.mult)                     # pick12
            out = []
            for ci in range(3):
                VC = n2_tile()
                tmp = n2_tile(sp)
                gtt(VC, c01[ci], G1, ALU.mult)
                gtt(tmp, c02[ci], G2, ALU.mult)
                gtt(VC, VC, tmp, ALU.add)
                gtt(tmp, c12[ci], G3, ALU.mult)
                gtt(VC, VC, tmp, ALU.add)
                out.append(VC)
            n2v = norm2(out)
            n = n2_tile(sp)
            polished_sqrt(n, n2v, T1D)
            nc.vector.tensor_scalar_max(n[:], n[:], 1e-30)
            nc.vector.reciprocal(n[:], n[:])
            for VC in out:
                gtt(VC, VC, n, ALU.mult)
            return out

        AXD = evec2(LL)
        XAX = [v[:, 0:NT] for v in AXD]
        ZAX = [v[:, NT:2 * NT] for v in AXD]

        NGHX = NGH[:, :, 0:K]
        NGHY = NGH[:, :, K:2 * K]
        NGHZ = NGH[:, :, 2 * K:3 * K]

        def axb(t_):
            return _ap(t_).rearrange("p t -> p t ()").to_broadcast([128, NT, K])

        def dot_axis(axes, g_engine=True):
            # batched NGH . axis over all tiles; products on gpsimd, adds on vec
            DST = cp.tile([128, NT, K], DT.float32, tag=f"dot{_tagn[0]}",
                          name=f"dot{_tagn[0]}")
            _tagn[0] += 1
            TA = sp.tile([128, NT, K], DT.float32, tag="dta")
            TB = sp.tile([128, NT, K], DT.float32, tag="dtb")
            eng = nc.gpsimd if g_engine else nc.vector
            eng.tensor_tensor(out=DST[:], in0=NGHX, in1=axb(axes[0]), op=ALU.mult)
            eng.tensor_tensor(out=TA[:], in0=NGHY, in1=axb(axes[1]), op=ALU.mult)
            eng.tensor_tensor(out=TB[:], in0=NGHZ, in1=axb(axes[2]), op=ALU.mult)
            nc.vector.tensor_tensor(out=DST[:], in0=DST[:], in1=TA[:], op=ALU.add)
            nc.vector.tensor_tensor(out=DST[:], in0=DST[:], in1=TB[:], op=ALU.add)
            return DST

        # ---- disambiguation dots + signs ----
        DOTX = dot_axis(XAX, g_engine=True)
        DOTZ = dot_axis(ZAX, g_engine=False)
        SG = cp.tile([128, NT, K], DT.float32)
        FX = nt_tile()
        FZ = nt_tile()
        for DOT, F in ((DOTX, FX), (DOTZ, FZ)):
            nc.scalar.activation(SG[:], DOT[:], AF.Sign)
            nc.vector.tensor_reduce(out=F[:], in_=SG[:], axis=AX.X, op=ALU.add)
            nc.scalar.activation(F[:], F[:], AF.Sign, bias=0.5, scale=1.0)
        for c in range(3):
            tt(XAX[c], XAX[c], FX, ALU.mult)
            tt(ZAX[c], ZAX[c], FZ, ALU.mult)
        nc.vector.tensor_tensor(out=DOTX[:], in0=DOTX[:], in1=axb(FX), op=ALU.mult)
        # y = cross(z, x)
        YAX = []
        for (i1, i2) in ((1, 2), (2, 0), (0, 1)):
            YC = nt_tile()
            YT = nt_tile(sp)
            gtt(YC, ZAX[i1], XAX[i2], ALU.mult)
            gtt(YT, ZAX[i2], XAX[i1], ALU.mult)
            gtt(YC, YC, YT, ALU.subtract)
            YAX.append(YC)
        DOTY = dot_axis(YAX, g_engine=True)

        # ---- projections (batched over all tiles) into PXY ----
        PXY = cp.tile([128, NT, 2, K], DT.float32)
        PXs = PXY[:, :, 0, :]
        PYs = PXY[:, :, 1, :]
        SC = cp.tile([128, NT, K], DT.float32)
        U2 = cp.tile([128, NT, K], DT.float32)
        RCN = cp.tile([128, NT, K], DT.float32)
        nc.scalar.activation(U2[:], DOTX[:], AF.Square)
        nc.scalar.activation(RCN[:], DOTY[:], AF.Square)
        nc.vector.tensor_tensor(out=U2[:], in0=U2[:], in1=RCN[:], op=ALU.add)
        nc.scalar.activation(SC[:], U2[:], AF.Sqrt)
        # one Newton step: s' = 0.5 (s + u/s) makes sqrt correctly-rounded-ish
        nc.vector.tensor_scalar_max(RCN[:], SC[:], 1e-30)
        nc.vector.reciprocal(RCN[:], RCN[:])
        nc.vector.tensor_tensor(out=RCN[:], in0=U2[:], in1=RCN[:], op=ALU.mult)
        nc.vector.tensor_tensor(out=SC[:], in0=SC[:], in1=RCN[:], op=ALU.add)
        nc.vector.tensor_scalar(out=SC[:], in0=SC[:], scalar1=0.5, scalar2=EPS,
                                op0=ALU.mult, op1=ALU.add)
        nc.vector.reciprocal(SC[:], SC[:])
        nc.vector.tensor_tensor(out=SC[:], in0=SC[:], in1=DD[:], op=ALU.mult)
        nc.vector.tensor_tensor(out=PXs, in0=DOTX[:], in1=SC[:], op=ALU.mult)
        nc.vector.tensor_tensor(out=PYs, in0=DOTY[:], in1=SC[:], op=ALU.mult)
        nc.sync.dma_start(pxy_o[:].rearrange("(t p) x k -> p t x k", p=128), PXY[:])

        # PPC = dd^2 + CKEY; with VM = -2r.U the key VAL = CKEY + d2d^2 - |t|^2
        # is strictly positive and fine-grained near the top (small) end
        PPC = cp.tile([128, NT, K], DT.float32)
        nc.scalar.activation(PPC[:], DD[:], AF.Square)
        nc.scalar.activation(PPC[:], PPC[:], AF.Identity, bias=CKEY, scale=1.0)

        # ---- BC selection, 2 v-tiles per step ----
        # Keys are positive (VAL = CKEY + d2d^2 - |t|^2 packed with k in the
        # low 5 bits); the 3 nearest are 3 rounds of segmented reduce-MIN.
        # The keep-mask is_le(KEY, min) doubles as the suppress addend
        # (+1.0 at the min pushes it above every real key < 0.25).
        # Emission is software-pipelined: gpsimd value-chains for a group of
        # steps first, then rounds pair-interleaved to hide cross-engine
        # latency (engine queues are in-order).
        TP = 2
        NS = NT // TP
        COSB = TCS[:, 0:8].rearrange("p a -> p () a ()") \
            .to_broadcast([128, TP, A, K])
        SINB = TCS[:, 8:16].rearrange("p a -> p () a ()") \
            .to_broadcast([128, TP, A, K])

        def emit_chain(st):
            t = st * TP
            pxb = PXY[:, t:t + TP, 0, :].rearrange("p t k -> p t () k") \
                .to_broadcast([128, TP, A, K])
            pyb = PXY[:, t:t + TP, 1, :].rearrange("p t k -> p t () k") \
                .to_broadcast([128, TP, A, K])
            T1B = bp.tile([128, TP, A, K], DT.float32, tag="t1b")
            T2B = bp.tile([128, TP, A, K], DT.float32, tag="t2b")
            nc.gpsimd.tensor_tensor(out=T1B[:], in0=pxb, in1=COSB, op=ALU.mult)
            nc.gpsimd.tensor_tensor(out=T2B[:], in0=pyb, in1=SINB, op=ALU.mult)
            UT = bp.tile([128, TP, A, K], DT.float32, tag="ut")
            nc.gpsimd.tensor_tensor(out=UT[:], in0=T1B[:], in1=T2B[:],
                                    op=ALU.add)
            # radius expansion on the (otherwise idle) scalar engine:
            # VALT[.., r-ring, ..] = -2 r_j . U
            VALT = bp.tile([128, TP, NCELL, K], DT.float32, tag="val", bufs=4)
            for r in range(R):
                nc.scalar.activation(
                    VALT[:, :, r * A:(r + 1) * A, :], UT[:], AF.Identity,
                    scale=RRJ_NEG2[r])
            ppcb = PPC[:, t:t + TP, :].rearrange("p t k -> p t () k") \
                .to_broadcast([128, TP, NCELL, K])
            nc.gpsimd.tensor_tensor(out=VALT[:], in0=VALT[:], in1=ppcb,
                                    op=ALU.add)
            return VALT

        def emit_select(st, VALT):
            # bit-pack in place (low 5 mantissa bits -> slot id, sign set),
            # then top-3 per cell directly via max8 — one op per cell, no
            # cross-op dependencies
            t = st * TP
            nc.vector.scalar_tensor_tensor(
                out=VALT[:].bitcast(DT.int32), in0=VALT[:].bitcast(DT.int32),
                scalar=M32[:], in1=KIOTA[:], op0=ALU.bitwise_and,
                op1=ALU.bitwise_or)
            M8 = bp.tile([128, TP, NCELL, 8], DT.float32, tag="m8", bufs=4)
            keyv = VALT[:].bitcast(DT.float32)
            for tt_ in range(TP):
                for c in range(NCELL):
                    nc.vector.max(out=M8[:, tt_, c, :], in_=keyv[:, tt_, c, :])
            M3C = bp.tile([128, TP, NCELL, 3], DT.float32, tag="m3c", bufs=4)
            nc.scalar.copy(M3C[:], M8[:, :, :, 0:3])
            nc.sync.dma_start(
                m3_o[t * 128:(t + TP) * 128, :, :]
                .rearrange("(t p) c s -> p t c s", p=128), M3C[:])

        # stream gp chains ahead; vector packs+max8s naturally pipeline
        states = {st: emit_chain(st) for st in range(3)}
        for st in range(NS):
            if st + 3 < NS:
                states[st + 3] = emit_chain(st + 3)
            emit_select(st, states[st])

    split_sync_waits(nc)
    return nc


# ---------------------------------------------------------------------------
# Host glue
# ---------------------------------------------------------------------------


def host_prep_phase1(vertices):
    """vertices (4, 4096, 3) -> list of 8 input maps."""
    maps = []
    for core in range(8):
        b, h = core // 2, core % 2
        verts = np.ascontiguousarray(vertices[b], dtype=f32)
        sq = (verts * verts).sum(-1, dtype=f32).astype(f32)
        pt4 = np.concatenate([verts.T, sq[None, :]], axis=0).astype(f32)
        Q = verts[h * HALF:(h + 1) * HALF]
        qt4 = np.concatenate([2.0 * Q.T, -np.ones((1, HALF), f32)],
                             axis=0).astype(f32)
        maps.append({"pt4": np.ascontiguousarray(pt4),
                     "qt4": np.ascontiguousarray(qt4)})
    return maps


_CHUNK_BASE = (np.arange(CAND, dtype=np.uint32) // 8 * 128)


def host_merge(cand_bits, verts, h):
    """Decode chunk candidates, exact fp32 top-33 re-rank.

    -> nbr (HALF,32) int64, d (HALF,32), radius (HALF,)."""
    bits = cand_bits.view(np.uint32)
    cand = ((bits & 127) + _CHUNK_BASE[None, :]).astype(np.int64)  # (HALF, 256)
    sq = (verts * verts).sum(-1, dtype=f32).astype(f32)
    Q = verts[h * HALF:(h + 1) * HALF]
    qsq = sq[h * HALF:(h + 1) * HALF]
    pc = verts[cand]                                               # (HALF,256,3)
    dots = np.einsum("qck,qk->qc", pc, Q, dtype=f32).astype(f32)
    d2 = (sq[cand] + qsq[:, None] - 2.0 * dots).astype(f32)
    order = np.lexsort((cand, d2), axis=1)[:, :33]
    top = np.take_along_axis(cand, order, axis=1)
    d2t = np.take_along_axis(d2, order, axis=1)
    d33 = np.sqrt(np.maximum(d2t, 0.0)).astype(f32)
    return top[:, :32], d33[:, :32], d33[:, 32]


def host_prep_phase2(vertices, template, p1_results):
    """Build phase-2 input maps + per-core nbr tables from phase-1 outputs."""
    template = np.asarray(template, f32)
    tx = template[..., 0]                    # (R, A)
    ty = template[..., 1]
    rr = tx[:, 0].astype(f32)                # angle 0: sin=0, cos=1 -> r_j
    assert np.array_equal(rr, _RRJ), "template radii differ from compiled-in"
    cosv = (tx[0] / rr[0]).astype(f32)
    sinv = (ty[0] / rr[0]).astype(f32)
    tcs = np.ascontiguousarray(np.broadcast_to(
        np.concatenate([cosv, sinv])[None, :], (128, 16))).astype(f32)
    rrn = np.zeros(8, f32)
    rrn[:R] = -2.0 * rr
    rrn = np.ascontiguousarray(np.broadcast_to(rrn[None, :], (128, 8))).astype(f32)
    maps, nbrs = [], []
    for core in range(8):
        b, h = core // 2, core % 2
        verts = np.ascontiguousarray(vertices[b], dtype=f32)
        nbr, d, radius = host_merge(p1_results[core]["cand"], verts, h)
        Q = verts[h * HALF:(h + 1) * HALF]
        neigh = (verts[nbr] - Q[:, None, :]).astype(f32)          # (HALF, 32, 3)
        ngh = np.ascontiguousarray(neigh.transpose(0, 2, 1).reshape(HALF, 96))
        w = (radius[:, None] - d).astype(f32)
        nw = (neigh * w[:, :, None]).astype(f32)
        cov = np.matmul(nw.transpose(0, 2, 1), neigh).astype(f32)  # (HALF, 3, 3)
        cov /= (w.sum(1, dtype=f32)[:, None, None] + f32(EPS))
        cov6 = np.zeros((HALF, 8), f32)
        cov6[:, 0] = cov[:, 0, 0]
        cov6[:, 1] = cov[:, 0, 1]
        cov6[:, 2] = cov[:, 0, 2]
        cov6[:, 3] = cov[:, 1, 1]
        cov6[:, 4] = cov[:, 1, 2]
        cov6[:, 5] = cov[:, 2, 2]
        maps.append({"ngh": ngh, "cov6": cov6, "dd": np.ascontiguousarray(d),
                     "tcs": tcs, "rrn": rrn})
        nbrs.append(nbr)
    return maps, nbrs


def host_assemble(p2_results, nbrs, template):
    """Decode k-slots, gather projections, barycentric weights, assemble."""
    template = np.asarray(template, f32)
    tmx = template[..., 0].reshape(NCELL).astype(f32)
    tmy = template[..., 1].reshape(NCELL).astype(f32)
    out = np.zeros((B, V, R, A, 3, 2), f32)
    rows = np.arange(HALF)[:, None, None]
    for core in range(8):
        b, h = core // 2, core % 2
        m3 = np.ascontiguousarray(p2_results[core]["m3o"])        # (HALF, 40, 3)
        k3 = (m3.view(np.int32) & 31).astype(np.int64)            # (HALF, 40, 3)
        pxy = p2_results[core]["pxy"]                             # (HALF, 2, 32)
        px = np.ascontiguousarray(pxy[:, 0, :])
        py = np.ascontiguousarray(pxy[:, 1, :])
        pxs = px[rows, k3]                                        # (HALF, 40, 3)
        pys = py[rows, k3]
        p0x, p1x, p2x = pxs[..., 0], pxs[..., 1], pxs[..., 2]
        p0y, p1y, p2y = pys[..., 0], pys[..., 1], pys[..., 2]
        v0x, v0y = p2x - p0x, p2y - p0y
        v1x, v1y = p1x - p0x, p1y - p0y
        v2x, v2y = tmx[None, :] - p0x, tmy[None, :] - p0y
        d00 = v0x * v0x + v0y * v0y
        d01 = v0x * v1x + v0y * v1y
        d02 = v0x * v2x + v0y * v2y
        d11 = v1x * v1x + v1y * v1y
        d12 = v1x * v2x + v1y * v2y
        den = d00 * d11 - d01 * d01 + f32(1e-6)
        w2 = (d11 * d02 - d01 * d12) / den
        w1 = (d00 * d12 - d01 * d02) / den
        w0 = f32(1.0) - w2 - w1
        weights = np.stack([w2, w1, w0], axis=-1)                 # (HALF, 40, 3)
        pidx = nbrs[core][rows[..., 0], k3.reshape(HALF, -1)].reshape(HALF, NCELL, 3)
        sl = slice(h * HALF, (h + 1) * HALF)
        out[b, sl, ..., 0] = pidx.reshape(HALF, R, A, 3).astype(f32)
        out[b, sl, ..., 1] = weights.reshape(HALF, R, A, 3).astype(f32)
    return out


_PROGS = {}


def _prog(name):
    if name not in _PROGS:
        _PROGS[name] = build_phase1() if name == "p1" else build_phase2()
    return _PROGS[name]


def run_phase1(vertices, trace=False):
    maps = host_prep_phase1(vertices)
    return run_bass_kernel_spmd(_prog("p1"), maps, list(range(8)), trace=trace)


def kernel(vertices, template, trace=False, _timing=None):
    vertices = np.asarray(vertices, f32)
    template = np.asarray(template, f32)
    r1 = run_bass_kernel_spmd(_prog("p1"), host_prep_phase1(vertices),
                              list(range(8)), trace=trace)
    maps2, nbrs = host_prep_phase2(vertices, template, r1.results)
    r2 = run_bass_kernel_spmd(_prog("p2"), maps2, list(range(8)), trace=trace)
    if _timing is not None:
        _timing["phase1"] = r1
        _timing["phase2"] = r2
        _timing["maps2"] = maps2
        _timing["nbrs"] = nbrs
    return host_assemble(r2.results, nbrs, template)


if __name__ == "__main__":
    # Phase-1 standalone check: exact top-33 coverage vs numpy brute force.
    cache = np.load("/root/problem/dev_cache/ref.npz")
    vertices = cache["vertices"]
    res = run_phase1(vertices)
    nbad = 0
    for core in range(8):
        b, h = core // 2, core % 2
        verts = np.ascontiguousarray(vertices[b], dtype=f32)
        nbr, d, rad = host_merge(res.results[core]["cand"], verts, h)
        # numpy exact reference
        sq = (verts * verts).sum(-1, dtype=f32).astype(f32)
        Q = verts[h * HALF:(h + 1) * HALF]
        d2full = (sq[None, :] + sq[h * HALF:(h + 1) * HALF, None]
                  - 2.0 * (Q @ verts.T)).astype(f32)
        order = np.lexsort((np.broadcast_to(np.arange(V), d2full.shape), d2full),
                           axis=1)[:, :33]
        miss = (np.sort(nbr, 1) != np.sort(order[:, :32], 1)).sum()
        print(f"core {core}: top32 mismatches={miss}")
        nbad += miss
    print("total nbr mismatches vs numpy exact:", nbad)


# revision 62
# speedup vs baseline: 1.1593x; 1.0010x over previous
"""Barycentric-coordinates KNN kernel for Trainium2 (8 NeuronCores).

Pipeline (per core = one (batch, half-of-V) pair; 8 cores cover 4 batches x 2
halves):
  Phase 1 (device): value matrix 2q.p - |p|^2 via fp32r TensorE matmuls
    (monotone in -d^2 per query row); column index bit-packed into the low 7
    mantissa bits; DVE max8 per 128-column chunk -> 256 candidate keys/row.
  Host: decode candidate indices, exact fp32 d^2 re-rank to the true top-33
    (value asc, index asc), neighbor gather, SHOT weight normalization.
  Phase 2 (device): weighted 3x3 covariance, closed-form eigensolver (Newton
    on the characteristic cubic + cross products), SHOT sign disambiguation,
    tangent-plane log map, template-cell top-3 selection via polar-factorized
    packed keys (VAL = C + dd^2 - 2 r.(px cos + py sin), low 5 bits = k) and
    3 rounds of segmented reduce-max + suppress.
  Host: decode k-slots, gather projections, barycentric weights, assemble
    (4, 4096, 5, 8, 3, 2) output.
"""
import sys

sys.path.insert(0, "/opt/trn_rl_repo")

import numpy as np
from contextlib import ExitStack

import concourse.bass as bass
import concourse.mybir as mybir
import concourse.tile as tile
from concourse.bass_utils import run_bass_kernel_spmd
from concourse.tile import ScopedClock

f32 = np.float32
AF = mybir.ActivationFunctionType
ALU = mybir.AluOpType
DT = mybir.dt
AX = mybir.AxisListType

B, V, K = 4, 4096, 32
HALF = V // 2            # queries per core
NT = HALF // 128         # 16 v-tiles per core
NCH = 32                 # phase-1 chunk count (chunk width 128)
CAND = NCH * 8           # 256 candidates per row
R, A = 5, 8
NCELL = R * A            # 40 template cells
EPS = 1e-8
CKEY = 0.015625          # key offset: VAL = CKEY + dd^2 - 2 p.t > 0
TEMPLATE_RADIUS = 0.09
# ring radii exactly as create_template computes them in fp32
_RRJ = (f32(TEMPLATE_RADIUS)
        * (np.arange(1, R + 1, dtype=f32) / f32(R))).astype(f32)
RRJ_NEG2 = [float(v) for v in (f32(-2.0) * _RRJ).astype(f32)]

# ---------------------------------------------------------------------------
# Tile-framework workaround: walrus rejects instructions carrying more than a
# couple of sync waits. Spread extras across single-wait NOPs.
# ---------------------------------------------------------------------------


def _patched_drain_and_barrier(self, tick_clock, wait_clock):
    probe = self.nc.sync.nop(nofuse=True)
    wait_clock.add_sem_waits(probe.ins, ScopedClock({None: tick_clock.global_clock}))
    sync_info = probe.ins.sync_info
    waits = list(sync_info.on_wait or []) if sync_info is not None else []
    if len(waits) > 1:
        sync_info.on_wait = waits[:1]
        for i in range(1, len(waits)):
            extra = self.nc.sync.nop(nofuse=True)
            if extra.ins.sync_info is None:
                extra.ins.sync_info = mybir.SyncInfo(on_wait=[waits[i]], on_update=[])
            else:
                extra.ins.sync_info.on_wait = [waits[i]]
    self.nc.sync.drain()
    self.nc.all_engine_barrier()
    assert self.sems is not None
    popped = self.nc._tile_sem_poison_stack.pop()
    assert popped is self._sem_poison
    self.nc.clear_and_free_semaphores(list(self.sems.allocated().values()))
    self.nc.all_engine_barrier()


tile.TileContext._drain_and_barrier = _patched_drain_and_barrier




def split_sync_waits(nc, max_waits=1):
    for f in nc.m.functions:
        for b in f.blocks:
            new_list = []
            dirty = False
            for ins in b.instructions:
                si = ins.sync_info
                waits = list(si.on_wait) if (si is not None and si.on_wait) else []
                if len(waits) > max_waits:
                    dirty = True
                    extras, keep = waits[:-max_waits], waits[-max_waits:]
                    for j in range(0, len(extras), max_waits):
                        nop = mybir.InstNoOp(
                            name=f"I-wsplit-{nc.next_id()}", engine=ins.engine
                        )
                        nop.sync_info = mybir.SyncInfo(
                            on_wait=extras[j : j + max_waits], on_update=[]
                        )
                        new_list.append(nop)
                    si.on_wait = keep
                new_list.append(ins)
            if dirty:
                b.instructions = new_list


# ---------------------------------------------------------------------------
# Phase 1 program
# ---------------------------------------------------------------------------


def build_phase1():
    nc = bass.Bass()
    pt4 = nc.declare_dram_parameter("pt4", [4, V], DT.float32r, isOutput=False)
    qt4 = nc.declare_dram_parameter("qt4", [4, HALF], DT.float32r, isOutput=False)
    cand_o = nc.declare_dram_parameter("cand", [HALF, CAND], DT.float32, isOutput=True)

    with tile.TileContext(nc) as tc, ExitStack() as ctx:
        cpool = ctx.enter_context(tc.tile_pool(name="const", bufs=1))
        kpool = ctx.enter_context(tc.tile_pool(name="keys", bufs=3))
        opool = ctx.enter_context(tc.tile_pool(name="cand", bufs=4))
        ppool = ctx.enter_context(tc.tile_pool(name="psum", bufs=2, space="PSUM"))

        pt = cpool.tile([4, V], DT.float32r)
        qt = cpool.tile([4, HALF], DT.float32r)
        nc.sync.dma_start(pt[:], pt4[:])
        nc.sync.dma_start(qt[:], qt4[:])
        # column-in-chunk index, repeated per chunk: 0..127, 0..127, ...
        kiota = cpool.tile([128, 2048], DT.int32)
        nc.gpsimd.iota(kiota[:], pattern=[[0, 16], [1, 128]], base=0,
                       channel_multiplier=0)
        m7 = cpool.tile([128, 1], DT.int32)
        nc.vector.memset(m7[:], -128)  # 0xFFFFFF80

        for t in range(NT):
            cv = opool.tile([128, CAND], DT.float32, tag="cv")
            for jh in range(2):
                ps = ppool.tile([128, 2048], DT.float32, space="PSUM")
                for k4 in range(4):
                    nc.tensor.matmul(
                        ps[:, k4 * 512:(k4 + 1) * 512],
                        qt[:, t * 128:(t + 1) * 128],
                        pt[:, jh * 2048 + k4 * 512: jh * 2048 + (k4 + 1) * 512],
                        start=True, stop=True,
                    )
                key = kpool.tile([128, 2048], DT.int32, tag="key", bufs=4)
                nc.vector.scalar_tensor_tensor(
                    out=key[:], in0=ps[:].bitcast(DT.int32), scalar=m7[:],
                    in1=kiota[:], op0=ALU.bitwise_and, op1=ALU.bitwise_or)
                for c in range(16):
                    g = jh * 16 + c
                    nc.vector.max(out=cv[:, g * 8:(g + 1) * 8],
                                  in_=key[:, c * 128:(c + 1) * 128]
                                  .bitcast(DT.float32))
            nc.sync.dma_start(cand_o[t * 128:(t + 1) * 128, :], cv[:])

    split_sync_waits(nc)
    return nc


# ---------------------------------------------------------------------------
# Phase 2 program
# ---------------------------------------------------------------------------


def _register_consts(nc, values):
    for value in values:
        t = nc.alloc_sbuf_tensor(f"const-float32-{value}", [128, 1], DT.float32)
        nc.gpsimd.memset(t.ap(), value)
        nc.const_aps.aps[(DT.float32, value)] = t.ap()
    nc.all_engine_barrier()


def build_phase2():
    nc = bass.Bass()
    _register_consts(nc, [0.5, CKEY, -3.0, 64.0])
    ngh_i = nc.declare_dram_parameter("ngh", [HALF, 96], DT.float32, isOutput=False)
    cov_i = nc.declare_dram_parameter("cov6", [HALF, 8], DT.float32, isOutput=False)
    dd_i = nc.declare_dram_parameter("dd", [HALF, K], DT.float32, isOutput=False)
    tcs_i = nc.declare_dram_parameter("tcs", [128, 16], DT.float32, isOutput=False)
    rrn_i = nc.declare_dram_parameter("rrn", [128, 8], DT.float32, isOutput=False)
    m3_o = nc.declare_dram_parameter("m3o", [HALF, NCELL, 3], DT.float32,
                                     isOutput=True)
    pxy_o = nc.declare_dram_parameter("pxy", [HALF, 2, K], DT.float32,
                                      isOutput=True)

    with tile.TileContext(nc) as tc, ExitStack() as ctx:
        cp = ctx.enter_context(tc.tile_pool(name="const", bufs=1))
        sp = ctx.enter_context(tc.tile_pool(name="scratch", bufs=2))
        bp = ctx.enter_context(tc.tile_pool(name="bc", bufs=2))

        NGH = cp.tile([128, NT, 96], DT.float32)
        COV6 = cp.tile([128, NT, 8], DT.float32)
        DD = cp.tile([128, NT, K], DT.float32)
        TCS = cp.tile([128, 16], DT.float32)
        RRN = cp.tile([128, 8], DT.float32)
        nc.sync.dma_start(NGH[:], ngh_i[:].rearrange("(t p) c -> p t c", p=128))
        nc.sync.dma_start(COV6[:], cov_i[:].rearrange("(t p) c -> p t c", p=128))
        nc.sync.dma_start(DD[:], dd_i[:].rearrange("(t p) c -> p t c", p=128))
        nc.sync.dma_start(TCS[:], tcs_i[:])
        nc.sync.dma_start(RRN[:], rrn_i[:])

        # low-5-bit slot id plus the sign bit: packed keys become negative
        # floats, so max8 ranks by ascending VAL with ties to the smaller k,
        # matching the reference tie-break
        KIOTA = cp.tile([128, 2, NCELL, K], DT.int32)
        nc.gpsimd.iota(KIOTA[:], pattern=[[0, 2], [0, NCELL], [1, K]],
                       base=-2147483648, channel_multiplier=0)
        M32 = cp.tile([128, 1], DT.int32)
        nc.vector.memset(M32[:], -32)

        _tagn = [0]

        def nt_tile(pool=cp):
            _tagn[0] += 1
            return pool.tile([128, NT], DT.float32, tag=f"nt{_tagn[0]}",
                             name=f"nt{_tagn[0]}")

        def n2_tile(pool=cp):
            _tagn[0] += 1
            return pool.tile([128, 2 * NT], DT.float32, tag=f"n2{_tagn[0]}",
                             name=f"n2{_tagn[0]}")

        CXX = COV6[:, :, 0]
        CXY = COV6[:, :, 1]
        CXZ = COV6[:, :, 2]
        CYY = COV6[:, :, 3]
        CYZ = COV6[:, :, 4]
        CZZ = COV6[:, :, 5]

        # ---- eigensolver; scalar chain on (128, NT), then the two Newton
        # runs and the two eigenvector extractions merged into (128, 2*NT) ----
        def _ap(x):
            return x if isinstance(x, bass.AP) else x[:]

        def tt(dst, a, bb, op):
            nc.vector.tensor_tensor(out=_ap(dst), in0=_ap(a), in1=_ap(bb), op=op)

        def sq_act(dst, a):
            nc.scalar.activation(_ap(dst), _ap(a), AF.Square)

        Q = nt_tile()
        tt(Q, CXX, CYY, ALU.add)
        tt(Q, Q, CZZ, ALU.add)
        nc.vector.tensor_scalar_mul(Q[:], Q[:], 1.0 / 3.0)
        BXX, BYY, BZZ = nt_tile(), nt_tile(), nt_tile()
        tt(BXX, CXX, Q, ALU.subtract)
        tt(BYY, CYY, Q, ALU.subtract)
        tt(BZZ, CZZ, Q, ALU.subtract)
        P2 = nt_tile()
        T1 = nt_tile(sp)
        sq_act(P2, BXX)
        sq_act(T1, BYY)
        tt(P2, P2, T1, ALU.add)
        sq_act(T1, BZZ)
        tt(P2, P2, T1, ALU.add)
        T2 = nt_tile(sp)
        sq_act(T1, CXY)
        sq_act(T2, CXZ)
        tt(T1, T1, T2, ALU.add)
        sq_act(T2, CYZ)
        tt(T1, T1, T2, ALU.add)
        nc.vector.tensor_scalar_mul(T1[:], T1[:], 2.0)
        tt(P2, P2, T1, ALU.add)
        PP = nt_tile()
        PPX = nt_tile()
        nc.vector.tensor_scalar_mul(PPX[:], P2[:], 1.0 / 6.0)

        def polished_sqrt(dst, x, tmp):
            # ACT Sqrt is ~7e-6; one Newton step s' = (s + x/s)/2 fixes it
            nc.scalar.activation(_ap(dst), _ap(x), AF.Sqrt)
            nc.vector.tensor_scalar_max(_ap(tmp), _ap(dst), 1e-30)
            nc.vector.reciprocal(_ap(tmp), _ap(tmp))
            nc.vector.tensor_tensor(out=_ap(tmp), in0=_ap(x), in1=_ap(tmp),
                                    op=ALU.mult)
            nc.vector.tensor_tensor(out=_ap(dst), in0=_ap(dst), in1=_ap(tmp),
                                    op=ALU.add)
            nc.vector.tensor_scalar_mul(_ap(dst), _ap(dst), 0.5)

        polished_sqrt(PP, PPX, T2)
        PINV = nt_tile()
        nc.vector.tensor_scalar_max(PINV[:], PP[:], 1e-20)
        nc.vector.reciprocal(PINV[:], PINV[:])
        NBXX, NBYY, NBZZ, NBXY, NBXZ, NBYZ = [nt_tile() for _ in range(6)]
        tt(NBXX, BXX, PINV, ALU.mult)
        tt(NBYY, BYY, PINV, ALU.mult)
        tt(NBZZ, BZZ, PINV, ALU.mult)
        tt(NBXY, CXY, PINV, ALU.mult)
        tt(NBXZ, CXZ, PINV, ALU.mult)
        tt(NBYZ, CYZ, PINV, ALU.mult)
        # det(B̂)
        DET = nt_tile()
        sq_act(T1, NBYZ)                     # byz^2
        tt(T2, NBYY, NBZZ, ALU.mult)
        tt(T2, T2, T1, ALU.subtract)
        tt(DET, NBXX, T2, ALU.mult)          # + bxx (byy bzz - byz^2)
        tt(T1, NBXY, NBZZ, ALU.mult)
        tt(T2, NBYZ, NBXZ, ALU.mult)
        tt(T1, T1, T2, ALU.subtract)
        tt(T1, NBXY, T1, ALU.mult)
        tt(DET, DET, T1, ALU.subtract)       # - bxy (bxy bzz - byz bxz)
        tt(T1, NBXY, NBYZ, ALU.mult)
        tt(T2, NBYY, NBXZ, ALU.mult)
        tt(T1, T1, T2, ALU.subtract)
        tt(T1, NBXZ, T1, ALU.mult)
        tt(DET, DET, T1, ALU.add)            # + bxz (bxy byz - byy bxz)
        R2 = nt_tile()                       # 2r = det  clamped to [-2, 2]
        nc.vector.tensor_scalar_min(R2[:], DET[:], 2.0)
        nc.vector.tensor_scalar_max(R2[:], R2[:], -2.0)

        # merged Newton: halves [beta(+2.2) | beta(-2.2)] over (128, 2*NT)
        def dup(src):
            d = n2_tile()
            nc.scalar.copy(d[:, 0:NT], _ap(src))
            nc.scalar.copy(d[:, NT:2 * NT], _ap(src))
            return d

        R2D = dup(R2)
        # [u | -u] for the odd-symmetry polynomial init: βmin(u) = -βmax(-u)
        R2S = n2_tile()
        nc.scalar.copy(R2S[:, 0:NT], R2[:])
        nc.scalar.activation(R2S[:, NT:2 * NT], R2[:], AF.Identity, scale=-1.0)
        T1D = n2_tile(sp)
        # cubic LS fit of the largest root of β³-3β-u on u ∈ [-2,2]
        # (max err 0.15), then 4 Newton steps -> <1e-6 away from double roots
        BETA = n2_tile()
        nc.vector.tensor_scalar(out=BETA[:], in0=R2S[:], scalar1=0.01574144,
                                scalar2=-0.03955863, op0=ALU.mult, op1=ALU.add)
        tt(BETA, BETA, R2S, ALU.mult)
        nc.vector.tensor_scalar_add(BETA[:], BETA[:], 0.15508261)
        tt(BETA, BETA, R2S, ALU.mult)
        nc.vector.tensor_scalar_add(BETA[:], BETA[:], 1.74024065)
        nc.scalar.activation(BETA[:, NT:2 * NT], BETA[:, NT:2 * NT],
                             AF.Identity, scale=-1.0)
        FV = n2_tile(sp)
        B2 = n2_tile(sp)
        for _ in range(4):
            tt(B2, BETA, BETA, ALU.mult)                  # β²
            tt(FV, B2, BETA, ALU.mult)                    # β³
            nc.vector.scalar_tensor_tensor(
                out=T1D[:], in0=BETA[:], scalar=3.0, in1=FV[:],
                op0=ALU.mult, op1=ALU.subtract)           # 3β - β³
            tt(T1D, T1D, R2D, ALU.add)                    # -f = 3β - β³ + 2r
            nc.vector.tensor_scalar(out=B2[:], in0=B2[:], scalar1=3.0,
                                    scalar2=-3.0, op0=ALU.mult, op1=ALU.add)
            nc.vector.tensor_scalar_max(B2[:], B2[:], 1e-8)
            nc.vector.reciprocal(B2[:], B2[:])
            tt(T1D, T1D, B2, ALU.mult)                    # -f/f'
            tt(BETA, BETA, T1D, ALU.add)                  # β - f/f'
        # LL = [λmax | λmin]
        PPD = dup(PP)
        QD = dup(Q)
        LL = n2_tile()
        tt(LL, PPD, BETA, ALU.mult)
        tt(LL, LL, QD, ALU.add)

        # merged eigenvector extraction: halves [x-axis(λmax) | z-axis(λmin)]
        CXXD, CXYD, CXZD = dup(CXX), dup(CXY), dup(CXZ)
        CYYD, CYZD, CZZD = dup(CYY), dup(CYZ), dup(CZZ)

        def gtt(dst, a, bb, op):
            nc.gpsimd.tensor_tensor(out=_ap(dst), in0=_ap(a), in1=_ap(bb), op=op)

        def evec2(lam):
            # columns of A - lam I; arithmetic on gpsimd (idle during the
            # eigen prologue), comparisons/reciprocals on vector, squares on
            # scalar
            D0, D1, D2 = n2_tile(sp), n2_tile(sp), n2_tile(sp)
            gtt(D0, CXXD, lam, ALU.subtract)
            gtt(D1, CYYD, lam, ALU.subtract)
            gtt(D2, CZZD, lam, ALU.subtract)
            m0 = (D0, CXYD, CXZD)
            m1 = (CXYD, D1, CYZD)
            m2 = (CXZD, CYZD, D2)

            def cross(u, v):
                rx, ry, rz = n2_tile(sp), n2_tile(sp), n2_tile(sp)
                tmp = n2_tile(sp)
                gtt(rx, u[1], v[2], ALU.mult)
                gtt(tmp, u[2], v[1], ALU.mult)
                gtt(rx, rx, tmp, ALU.subtract)
                gtt(ry, u[2], v[0], ALU.mult)
                gtt(tmp, u[0], v[2], ALU.mult)
                gtt(ry, ry, tmp, ALU.subtract)
                gtt(rz, u[0], v[1], ALU.mult)
                gtt(tmp, u[1], v[0], ALU.mult)
                gtt(rz, rz, tmp, ALU.subtract)
                return rx, ry, rz

            def norm2(c):
                n = n2_tile(sp)
                tmp = n2_tile(sp)
                sq_act(n, c[0])
                sq_act(tmp, c[1])
                gtt(n, n, tmp, ALU.add)
                sq_act(tmp, c[2])
                gtt(n, n, tmp, ALU.add)
                return n

            c01 = cross(m0, m1)
            c02 = cross(m0, m2)
            c12 = cross(m1, m2)
            n01, n02, n12 = norm2(c01), norm2(c02), norm2(c12)
            G1, G2, G3 = n2_tile(sp), n2_tile(sp), n2_tile(sp)
            tt(G1, n01, n02, ALU.is_ge)
            tt(G2, n01, n12, ALU.is_ge)
            tt(G1, G1, G2, ALU.mult)                    # pick01
            tt(G3, n02, n12, ALU.is_ge)
            U = n2_tile(sp)
            nc.vector.tensor_scalar(out=U[:], in0=G1[:], scalar1=-1.0, scalar2=1.0,
                                    op0=ALU.mult, op1=ALU.add)   # 1 - pick01
            tt(G2, U, G3, ALU.mult)                     # pick02
            nc.vector.tensor_scalar(out=G3[:], in0=G3[:], scalar1=-1.0, scalar2=1.0,
                                    op0=ALU.mult, op1=ALU.add)   # 1 - g3
            tt(G3, U, G3, ALU.mult)                     # pick12
            out = []
            for ci in range(3):
                VC = n2_tile()
                tmp = n2_tile(sp)
                gtt(VC, c01[ci], G1, ALU.mult)
                gtt(tmp, c02[ci], G2, ALU.mult)
                gtt(VC, VC, tmp, ALU.add)
                gtt(tmp, c12[ci], G3, ALU.mult)
                gtt(VC, VC, tmp, ALU.add)
                out.append(VC)
            n2v = norm2(out)
            n = n2_tile(sp)
            polished_sqrt(n, n2v, T1D)
            nc.vector.tensor_scalar_max(n[:], n[:], 1e-30)
            nc.vector.reciprocal(n[:], n[:])
            for VC in out:
                gtt(VC, VC, n, ALU.mult)
            return out

        AXD = evec2(LL)
        XAX = [v[:, 0:NT] for v in AXD]
        ZAX = [v[:, NT:2 * NT] for v in AXD]

        NGHX = NGH[:, :, 0:K]
        NGHY = NGH[:, :, K:2 * K]
        NGHZ = NGH[:, :, 2 * K:3 * K]

        def axb(t_):
            return _ap(t_).rearrange("p t -> p t ()").to_broadcast([128, NT, K])

        def dot_axis(axes, g_engine=True):
            # batched NGH . axis over all tiles; products on gpsimd, adds on vec
            DST = cp.tile([128, NT, K], DT.float32, tag=f"dot{_tagn[0]}",
                          name=f"dot{_tagn[0]}")
            _tagn[0] += 1
            TA = sp.tile([128, NT, K], DT.float32, tag="dta")
            TB = sp.tile([128, NT, K], DT.float32, tag="dtb")
            eng = nc.gpsimd if g_engine else nc.vector
            eng.tensor_tensor(out=DST[:], in0=NGHX, in1=axb(axes[0]), op=ALU.mult)
            eng.tensor_tensor(out=TA[:], in0=NGHY, in1=axb(axes[1]), op=ALU.mult)
            eng.tensor_tensor(out=TB[:], in0=NGHZ, in1=axb(axes[2]), op=ALU.mult)
            nc.vector.tensor_tensor(out=DST[:], in0=DST[:], in1=TA[:], op=ALU.add)
            nc.vector.tensor_tensor(out=DST[:], in0=DST[:], in1=TB[:], op=ALU.add)
            return DST

        # ---- disambiguation dots + signs ----
        DOTX = dot_axis(XAX, g_engine=True)
        DOTZ = dot_axis(ZAX, g_engine=True)
        SG = cp.tile([128, NT, K], DT.float32)
        FX = nt_tile()
        FZ = nt_tile()
        for DOT, F in ((DOTX, FX), (DOTZ, FZ)):
            nc.scalar.activation(SG[:], DOT[:], AF.Sign)
            nc.vector.tensor_reduce(out=F[:], in_=SG[:], axis=AX.X, op=ALU.add)
            nc.scalar.activation(F[:], F[:], AF.Sign, bias=0.5, scale=1.0)
        for c in range(3):
            tt(XAX[c], XAX[c], FX, ALU.mult)
            tt(ZAX[c], ZAX[c], FZ, ALU.mult)
        nc.vector.tensor_tensor(out=DOTX[:], in0=DOTX[:], in1=axb(FX), op=ALU.mult)
        # y = cross(z, x)
        YAX = []
        for (i1, i2) in ((1, 2), (2, 0), (0, 1)):
            YC = nt_tile()
            YT = nt_tile(sp)
            gtt(YC, ZAX[i1], XAX[i2], ALU.mult)
            gtt(YT, ZAX[i2], XAX[i1], ALU.mult)
            gtt(YC, YC, YT, ALU.subtract)
            YAX.append(YC)
        DOTY = dot_axis(YAX, g_engine=True)

        # ---- projections (batched over all tiles) into PXY ----
        PXY = cp.tile([128, NT, 2, K], DT.float32)
        PXs = PXY[:, :, 0, :]
        PYs = PXY[:, :, 1, :]
        SC = cp.tile([128, NT, K], DT.float32)
        U2 = cp.tile([128, NT, K], DT.float32)
        RCN = cp.tile([128, NT, K], DT.float32)
        nc.scalar.activation(U2[:], DOTX[:], AF.Square)
        nc.scalar.activation(RCN[:], DOTY[:], AF.Square)
        nc.vector.tensor_tensor(out=U2[:], in0=U2[:], in1=RCN[:], op=ALU.add)
        nc.scalar.activation(SC[:], U2[:], AF.Sqrt)
        # one Newton step: s' = 0.5 (s + u/s) makes sqrt correctly-rounded-ish
        nc.vector.tensor_scalar_max(RCN[:], SC[:], 1e-30)
        nc.vector.reciprocal(RCN[:], RCN[:])
        nc.vector.tensor_tensor(out=RCN[:], in0=U2[:], in1=RCN[:], op=ALU.mult)
        nc.vector.tensor_tensor(out=SC[:], in0=SC[:], in1=RCN[:], op=ALU.add)
        nc.vector.tensor_scalar(out=SC[:], in0=SC[:], scalar1=0.5, scalar2=EPS,
                                op0=ALU.mult, op1=ALU.add)
        nc.vector.reciprocal(SC[:], SC[:])
        nc.vector.tensor_tensor(out=SC[:], in0=SC[:], in1=DD[:], op=ALU.mult)
        nc.vector.tensor_tensor(out=PXs, in0=DOTX[:], in1=SC[:], op=ALU.mult)
        nc.vector.tensor_tensor(out=PYs, in0=DOTY[:], in1=SC[:], op=ALU.mult)
        nc.sync.dma_start(pxy_o[:].rearrange("(t p) x k -> p t x k", p=128), PXY[:])

        # PPC = dd^2 + CKEY; with VM = -2r.U the key VAL = CKEY + d2d^2 - |t|^2
        # is strictly positive and fine-grained near the top (small) end
        PPC = cp.tile([128, NT, K], DT.float32)
        nc.scalar.activation(PPC[:], DD[:], AF.Square)
        nc.scalar.activation(PPC[:], PPC[:], AF.Identity, bias=CKEY, scale=1.0)

        # ---- BC selection, 2 v-tiles per step ----
        # Keys are positive (VAL = CKEY + d2d^2 - |t|^2 packed with k in the
        # low 5 bits); the 3 nearest are 3 rounds of segmented reduce-MIN.
        # The keep-mask is_le(KEY, min) doubles as the suppress addend
        # (+1.0 at the min pushes it above every real key < 0.25).
        # Emission is software-pipelined: gpsimd value-chains for a group of
        # steps first, then rounds pair-interleaved to hide cross-engine
        # latency (engine queues are in-order).
        TP = 2
        NS = NT // TP
        COSB = TCS[:, 0:8].rearrange("p a -> p () a ()") \
            .to_broadcast([128, TP, A, K])
        SINB = TCS[:, 8:16].rearrange("p a -> p () a ()") \
            .to_broadcast([128, TP, A, K])

        def emit_chain(st):
            t = st * TP
            pxb = PXY[:, t:t + TP, 0, :].rearrange("p t k -> p t () k") \
                .to_broadcast([128, TP, A, K])
            pyb = PXY[:, t:t + TP, 1, :].rearrange("p t k -> p t () k") \
                .to_broadcast([128, TP, A, K])
            T1B = bp.tile([128, TP, A, K], DT.float32, tag="t1b")
            T2B = bp.tile([128, TP, A, K], DT.float32, tag="t2b")
            nc.gpsimd.tensor_tensor(out=T1B[:], in0=pxb, in1=COSB, op=ALU.mult)
            nc.gpsimd.tensor_tensor(out=T2B[:], in0=pyb, in1=SINB, op=ALU.mult)
            UT = bp.tile([128, TP, A, K], DT.float32, tag="ut")
            nc.gpsimd.tensor_tensor(out=UT[:], in0=T1B[:], in1=T2B[:],
                                    op=ALU.add)
            # radius expansion on the (otherwise idle) scalar engine:
            # VALT[.., r-ring, ..] = -2 r_j . U
            VALT = bp.tile([128, TP, NCELL, K], DT.float32, tag="val", bufs=5)
            for r in range(R):
                nc.scalar.activation(
                    VALT[:, :, r * A:(r + 1) * A, :], UT[:], AF.Identity,
                    scale=RRJ_NEG2[r])
            ppcb = PPC[:, t:t + TP, :].rearrange("p t k -> p t () k") \
                .to_broadcast([128, TP, NCELL, K])
            nc.gpsimd.tensor_tensor(out=VALT[:], in0=VALT[:], in1=ppcb,
                                    op=ALU.add)
            return VALT

        def emit_select(st, VALT):
            # bit-pack in place (low 5 mantissa bits -> slot id, sign set),
            # then top-3 per cell directly via max8 — one op per cell, no
            # cross-op dependencies
            t = st * TP
            nc.vector.scalar_tensor_tensor(
                out=VALT[:].bitcast(DT.int32), in0=VALT[:].bitcast(DT.int32),
                scalar=M32[:], in1=KIOTA[:], op0=ALU.bitwise_and,
                op1=ALU.bitwise_or)
            M8 = bp.tile([128, TP, NCELL, 8], DT.float32, tag="m8", bufs=6)
            keyv = VALT[:].bitcast(DT.float32)
            for tt_ in range(TP):
                for c in range(NCELL):
                    nc.vector.max(out=M8[:, tt_, c, :], in_=keyv[:, tt_, c, :])
            M3C = bp.tile([128, TP, NCELL, 3], DT.float32, tag="m3c", bufs=4)
            nc.scalar.copy(M3C[:], M8[:, :, :, 0:3])
            nc.sync.dma_start(
                m3_o[t * 128:(t + TP) * 128, :, :]
                .rearrange("(t p) c s -> p t c s", p=128), M3C[:])

        # stream gp chains ahead; vector packs+max8s naturally pipeline
        states = {st: emit_chain(st) for st in range(3)}
        for st in range(NS):
            if st + 3 < NS:
                states[st + 3] = emit_chain(st + 3)
            emit_select(st, states[st])

    split_sync_waits(nc)
    return nc


# ---------------------------------------------------------------------------
# Host glue
# ---------------------------------------------------------------------------


def host_prep_phase1(vertices):
    """vertices (4, 4096, 3) -> list of 8 input maps."""
    maps = []
    for core in range(8):
        b, h = core // 2, core % 2
        verts = np.ascontiguousarray(vertices[b], dtype=f32)
        sq = (verts * verts).sum(-1, dtype=f32).astype(f32)
        pt4 = np.concatenate([verts.T, sq[None, :]], axis=0).astype(f32)
        Q = verts[h * HALF:(h + 1) * HALF]
        qt4 = np.concatenate([2.0 * Q.T, -np.ones((1, HALF), f32)],
                             axis=0).astype(f32)
        maps.append({"pt4": np.ascontiguousarray(pt4),
                     "qt4": np.ascontiguousarray(qt4)})
    return maps


_CHUNK_BASE = (np.arange(CAND, dtype=np.uint32) // 8 * 128)


def host_merge(cand_bits, verts, h):
    """Decode chunk candidates, exact fp32 top-33 re-rank.

    -> nbr (HALF,32) int64, d (HALF,32), radius (HALF,)."""
    bits = cand_bits.view(np.uint32)
    cand = ((bits & 127) + _CHUNK_BASE[None, :]).astype(np.int64)  # (HALF, 256)
    sq = (verts * verts).sum(-1, dtype=f32).astype(f32)
    Q = verts[h * HALF:(h + 1) * HALF]
    qsq = sq[h * HALF:(h + 1) * HALF]
    pc = verts[cand]                                               # (HALF,256,3)
    dots = np.einsum("qck,qk->qc", pc, Q, dtype=f32).astype(f32)
    d2 = (sq[cand] + qsq[:, None] - 2.0 * dots).astype(f32)
    order = np.lexsort((cand, d2), axis=1)[:, :33]
    top = np.take_along_axis(cand, order, axis=1)
    d2t = np.take_along_axis(d2, order, axis=1)
    d33 = np.sqrt(np.maximum(d2t, 0.0)).astype(f32)
    return top[:, :32], d33[:, :32], d33[:, 32]


def host_prep_phase2(vertices, template, p1_results):
    """Build phase-2 input maps + per-core nbr tables from phase-1 outputs."""
    template = np.asarray(template, f32)
    tx = template[..., 0]                    # (R, A)
    ty = template[..., 1]
    rr = tx[:, 0].astype(f32)                # angle 0: sin=0, cos=1 -> r_j
    assert np.array_equal(rr, _RRJ), "template radii differ from compiled-in"
    cosv = (tx[0] / rr[0]).astype(f32)
    sinv = (ty[0] / rr[0]).astype(f32)
    tcs = np.ascontiguousarray(np.broadcast_to(
        np.concatenate([cosv, sinv])[None, :], (128, 16))).astype(f32)
    rrn = np.zeros(8, f32)
    rrn[:R] = -2.0 * rr
    rrn = np.ascontiguousarray(np.broadcast_to(rrn[None, :], (128, 8))).astype(f32)
    maps, nbrs = [], []
    for core in range(8):
        b, h = core // 2, core % 2
        verts = np.ascontiguousarray(vertices[b], dtype=f32)
        nbr, d, radius = host_merge(p1_results[core]["cand"], verts, h)
        Q = verts[h * HALF:(h + 1) * HALF]
        neigh = (verts[nbr] - Q[:, None, :]).astype(f32)          # (HALF, 32, 3)
        ngh = np.ascontiguousarray(neigh.transpose(0, 2, 1).reshape(HALF, 96))
        w = (radius[:, None] - d).astype(f32)
        nw = (neigh * w[:, :, None]).astype(f32)
        cov = np.matmul(nw.transpose(0, 2, 1), neigh).astype(f32)  # (HALF, 3, 3)
        cov /= (w.sum(1, dtype=f32)[:, None, None] + f32(EPS))
        cov6 = np.zeros((HALF, 8), f32)
        cov6[:, 0] = cov[:, 0, 0]
        cov6[:, 1] = cov[:, 0, 1]
        cov6[:, 2] = cov[:, 0, 2]
        cov6[:, 3] = cov[:, 1, 1]
        cov6[:, 4] = cov[:, 1, 2]
        cov6[:, 5] = cov[:, 2, 2]
        maps.append({"ngh": ngh, "cov6": cov6, "dd": np.ascontiguousarray(d),
                     "tcs": tcs, "rrn": rrn})
        nbrs.append(nbr)
    return maps, nbrs


def host_assemble(p2_results, nbrs, template):
    """Decode k-slots, gather projections, barycentric weights, assemble."""
    template = np.asarray(template, f32)
    tmx = template[..., 0].reshape(NCELL).astype(f32)
    tmy = template[..., 1].reshape(NCELL).astype(f32)
    out = np.zeros((B, V, R, A, 3, 2), f32)
    rows = np.arange(HALF)[:, None, None]
    for core in range(8):
        b, h = core // 2, core % 2
        m3 = np.ascontiguousarray(p2_results[core]["m3o"])        # (HALF, 40, 3)
        k3 = (m3.view(np.int32) & 31).astype(np.int64)            # (HALF, 40, 3)
        pxy = p2_results[core]["pxy"]                             # (HALF, 2, 32)
        px = np.ascontiguousarray(pxy[:, 0, :])
        py = np.ascontiguousarray(pxy[:, 1, :])
        pxs = px[rows, k3]                                        # (HALF, 40, 3)
        pys = py[rows, k3]
        p0x, p1x, p2x = pxs[..., 0], pxs[..., 1], pxs[..., 2]
        p0y, p1y, p2y = pys[..., 0], pys[..., 1], pys[..., 2]
        v0x, v0y = p2x - p0x, p2y - p0y
        v1x, v1y = p1x - p0x, p1y - p0y
        v2x, v2y = tmx[None, :] - p0x, tmy[None, :] - p0y
        d00 = v0x * v0x + v0y * v0y
        d01 = v0x * v1x + v0y * v1y
        d02 = v0x * v2x + v0y * v2y
        d11 = v1x * v1x + v1y * v1y
        d12 = v1x * v2x + v1y * v2y
        den = d00 * d11 - d01 * d01 + f32(1e-6)
        w2 = (d11 * d02 - d01 * d12) / den
        w1 = (d00 * d12 - d01 * d02) / den
        w0 = f32(1.0) - w2 - w1
        weights = np.stack([w2, w1, w0], axis=-1)                 # (HALF, 40, 3)
        pidx = nbrs[core][rows[..., 0], k3.reshape(HALF, -1)].reshape(HALF, NCELL, 3)
        sl = slice(h * HALF, (h + 1) * HALF)
        out[b, sl, ..., 0] = pidx.reshape(HALF, R, A, 3).astype(f32)
        out[b, sl, ..., 1] = weights.reshape(HALF, R, A, 3).astype(f32)
    return out


_PROGS = {}


def _prog(name):
    if name not in _PROGS:
        _PROGS[name] = build_phase1() if name == "p1" else build_phase2()
    return _PROGS[name]


def run_phase1(vertices, trace=False):
    maps = host_prep_phase1(vertices)
    return run_bass_kernel_spmd(_prog("p1"), maps, list(range(8)), trace=trace)


def kernel(vertices, template, trace=False, _timing=None):
    vertices = np.asarray(vertices, f32)
    template = np.asarray(template, f32)
    r1 = run_bass_kernel_spmd(_prog("p1"), host_prep_phase1(vertices),
                              list(range(8)), trace=trace)
    maps2, nbrs = host_prep_phase2(vertices, template, r1.results)
    r2 = run_bass_kernel_spmd(_prog("p2"), maps2, list(range(8)), trace=trace)
    if _timing is not None:
        _timing["phase1"] = r1
        _timing["phase2"] = r2
        _timing["maps2"] = maps2
        _timing["nbrs"] = nbrs
    return host_assemble(r2.results, nbrs, template)


if __name__ == "__main__":
    # Phase-1 standalone check: exact top-33 coverage vs numpy brute force.
    cache = np.load("/root/problem/dev_cache/ref.npz")
    vertices = cache["vertices"]
    res = run_phase1(vertices)
    nbad = 0
    for core in range(8):
        b, h = core // 2, core % 2
        verts = np.ascontiguousarray(vertices[b], dtype=f32)
        nbr, d, rad = host_merge(res.results[core]["cand"], verts, h)
        # numpy exact reference
        sq = (verts * verts).sum(-1, dtype=f32).astype(f32)
        Q = verts[h * HALF:(h + 1) * HALF]
        d2full = (sq[None, :] + sq[h * HALF:(h + 1) * HALF, None]
                  - 2.0 * (Q @ verts.T)).astype(f32)
        order = np.lexsort((np.broadcast_to(np.arange(V), d2full.shape), d2full),
                           axis=1)[:, :33]
        miss = (np.sort(nbr, 1) != np.sort(order[:, :32], 1)).sum()
        print(f"core {core}: top32 mismatches={miss}")
        nbad += miss
    print("total nbr mismatches vs numpy exact:", nbad)
